# revision 8
# baseline (speedup 1.0000x reference)
"""Trainium2 Bass kernel for nn_Decoder_20486994002617.

8-core tensor-parallel 2-layer llama-style decoder with ragged token-merge
(handled on host), returning the masked-mean cross-entropy loss.

Device layout choices:
  - h (residual) lives in SBUF as [128 part, 8 seq-tiles, 4096] bf16.
  - RMSNorm weights are folded into the consumer weight matrices on host,
    so the device norm is x * rsqrt(mean(x^2)+eps) only; the multiply by
    the per-row factor is fused into the seq->feature transpose as a
    matmul against diag(factor).
  - Attention: heads sharded 4 q-heads + 1 kv-head per core (GQA groups
    align), scores/softmax per (head, 128-row tile), causal mask added via
    an extra accumulating matmul (I.T @ cmask), attn probs transposed back
    through the PE with diag(1/sumexp) fused.
  - MLP: intermediate dim sharded 1376/core, padded to 1408 = 11*128.
  - lm_head: vocab sharded 4000/core; softmax stats (row max, sum-exp) are
    AllReduce'd; the target logit is computed via a host-gathered column
    matrix (wsel) so no device gather is needed.
Outputs per core: gmax [128,8] f32, gsum [128,8] f32, tlog [1,1024] f32.
Host finishes: ce = gmax + log(gsum) - tlog; loss = masked mean.
"""
import numpy as np
import ml_dtypes

from contextlib import ExitStack

import concourse.bass as bass
import concourse.bacc as bacc
import concourse.mybir as mybir
import concourse.tile as tile
from concourse.bass_utils import run_bass_kernel_spmd

F32 = mybir.dt.float32
BF16 = mybir.dt.bfloat16
AF = mybir.ActivationFunctionType
ALU = mybir.AluOpType
AX = mybir.AxisListType

H, HD, NH, NKV = 4096, 128, 32, 8
L, V, S, I = 2, 32000, 1024, 11008
EPS, THETA = 1e-6, 10000.0
NC_ = 8          # cores
IPC = I // NC_   # 1376
IP = 1408        # padded intermediate per core = 11 * 128
VS = V // NC_    # 4000 vocab per core
NEG = -1e9

bf16 = ml_dtypes.bfloat16

last_run_info = {}
_cache = {}


# ----------------------------------------------------------------- device --

def _norm_transpose(nc, pools, h_ap, dst, ident_sb, uid):
    """dst[:, k, :] (32 chunks of [128,128]) = normalized transpose of
    h_ap ([128 seq rows, 4096]). dst free dims must be (32, 128)."""
    small, ntmp, psum = pools
    sq = ntmp.tile([128, 4096], BF16, tag="nt_sq", bufs=1, name=f"sq_{uid}")
    ssq = small.tile([128, 1], F32, tag="nt_ssq", bufs=2, name=f"ssq_{uid}")
    nc.scalar.activation(sq[:], h_ap, AF.Square, accum_out=ssq[:])
    var = small.tile([128, 1], F32, tag="nt_var", bufs=2, name=f"var_{uid}")
    nc.vector.tensor_scalar(var[:], ssq[:], 1.0 / H, EPS, op0=ALU.mult, op1=ALU.add)
    std = small.tile([128, 1], F32, tag="nt_std", bufs=2, name=f"std_{uid}")
    nc.scalar.sqrt(std[:], var[:])
    fac = small.tile([128, 1], F32, tag="nt_fac", bufs=2, name=f"fac_{uid}")
    nc.vector.reciprocal(fac[:], std[:])
    diag = ntmp.tile([128, 128], BF16, tag="nt_diag", bufs=2, name=f"diag_{uid}")
    nc.vector.tensor_scalar_mul(diag[:], ident_sb[:], fac[:])
    for kk in range(8):
        pnt = psum.tile([128, 512], F32, tag="nt_ps", bufs=2, name=f"pnt_{uid}_{kk}")
        for j in range(4):
            k = kk * 4 + j
            nc.tensor.matmul(pnt[:, j * 128:(j + 1) * 128],
                             h_ap[:, k * 128:(k + 1) * 128], diag[:],
                             start=True, stop=True)
        nc.any.tensor_copy(dst[:, kk * 4:(kk + 1) * 4, :],
                           pnt[:].rearrange("p (j m) -> p j m", j=4))


def _rope(nc, pools, ps, cos_ap, sf_ap, out, nheads, i):
    """out (bf16 [128, nheads*128]) = rope(ps) with ps a psum slice."""
    small, ntmp, psum = pools
    n = nheads * 128
    t1 = ntmp.tile([128, 512], F32, tag="rope_t1", bufs=2, name=f"t1_{i}_{nheads}")
    t2 = ntmp.tile([128, 512], F32, tag="rope_t2", bufs=2, name=f"t2_{i}_{nheads}")
    nc.vector.tensor_mul(t1[:, :n], ps, cos_ap)
    for hh in range(nheads):
        b = hh * 128
        nc.vector.tensor_mul(t2[:, b:b + 64], ps[:, b + 64:b + 128],
                             sf_ap[:, b:b + 64])
        nc.vector.tensor_mul(t2[:, b + 64:b + 128], ps[:, b:b + 64],
                             sf_ap[:, b + 64:b + 128])
    nc.vector.tensor_add(out[:], t1[:, :n], t2[:, :n])


def build_nc():
    nc = bacc.Bacc("TRN2", target_bir_lowering=False, debug=False,
                   num_devices=NC_)

    din = {}
    def dram_in(name, shape):
        din[name] = nc.dram_tensor(name, shape, BF16, kind="ExternalInput")
        return din[name]

    h0_d = dram_in("h0", [S, H])
    cos4_d = dram_in("cos4", [S, 512])
    sf4_d = dram_in("sf4", [S, 512])
    ident_d = dram_in("ident", [128, 128])
    cmask_d = dram_in("cmask", [128, 128])
    ones_d = dram_in("ones", [128, 1])
    for l in range(L):
        dram_in(f"qw{l}", [H, 512])
        dram_in(f"kvw{l}", [H, 256])
        dram_in(f"ow{l}", [512, H])
        dram_in(f"gw{l}", [H, IP])
        dram_in(f"uw{l}", [H, IP])
        dram_in(f"dw{l}", [IP, H])
    lmw_d = dram_in("lmw", [8, H, VS // 8])
    wsel_d = dram_in("wsel", [H, S])

    gmax_o = nc.dram_tensor("gmax_o", [128, 8], F32, kind="ExternalOutput")
    gsum_o = nc.dram_tensor("gsum_o", [128, 8], F32, kind="ExternalOutput")
    tlog_o = nc.dram_tensor("tlog_o", [1, S], F32, kind="ExternalOutput")

    rg = [list(range(NC_))]

    with tile.TileContext(nc) as tc:
        with (
            tc.tile_pool(name="pconst", bufs=1) as pconst,
            tc.tile_pool(name="psmall", bufs=1) as psmall,
            tc.tile_pool(name="pdram", bufs=1, space="DRAM") as pdram,
        ):
            ident_sb = pconst.tile([128, 128], BF16)
            cmask_sb = pconst.tile([128, 128], BF16)
            ones_sb = pconst.tile([128, 1], BF16)
            cos4_sb = pconst.tile([128, 8, 512], BF16)
            sf4_sb = pconst.tile([128, 8, 512], BF16)
            nc.sync.dma_start(ident_sb[:], ident_d.ap())
            nc.sync.dma_start(cmask_sb[:], cmask_d.ap())
            nc.sync.dma_start(ones_sb[:], ones_d.ap())
            for i in range(8):
                nc.sync.dma_start(cos4_sb[:, i, :], cos4_d.ap()[i * 128:(i + 1) * 128, :])
                nc.sync.dma_start(sf4_sb[:, i, :], sf4_d.ap()[i * 128:(i + 1) * 128, :])

            hstack = ExitStack()
            phh = hstack.enter_context(tc.tile_pool(name="phh", bufs=1))
            if True:
                h_sb = phh.tile([128, 8, H], BF16)
                for i in range(8):
                    nc.sync.dma_start(h_sb[:, i, :], h0_d.ap()[i * 128:(i + 1) * 128, :])

                for l in range(L):
                    # ============================== attention ==============
                    with (
                        tc.tile_pool(name="pal", bufs=1) as pal,      # attn-long
                        tc.tile_pool(name="pdr", bufs=1, space="DRAM") as pdr,
                    ):
                        qT_sb = pal.tile([128, 4, S], BF16)
                        kT_sb = pal.tile([128, S], BF16)
                        v_sb = pal.tile([128, 8, 128], BF16)
                        oT_sb = pal.tile([128, 4, S], BF16)
                        ar_in = pdr.tile([S, H], BF16)
                        ar_out = pdr.tile([S, H], BF16, addr_space="Shared")

                        with (
                            tc.tile_pool(name="pqkv", bufs=1) as pqkv,
                            tc.tile_pool(name="pqps", bufs=1, space="PSUM") as pqps,
                        ):
                            pools = (psmall, pqkv, pqps)
                            wq_sb = pqkv.tile([128, 32, 512], BF16)
                            wkv_sb = pqkv.tile([128, 32, 256], BF16)
                            nc.sync.dma_start(
                                wq_sb[:], din[f"qw{l}"].ap().rearrange("(k p) n -> p k n", p=128))
                            nc.sync.dma_start(
                                wkv_sb[:], din[f"kvw{l}"].ap().rearrange("(k p) n -> p k n", p=128))
                            for i in range(8):
                                xnt = pqkv.tile([128, 32, 128], BF16, tag="xnt",
                                                bufs=2, name=f"xnt_{l}_{i}")
                                _norm_transpose(nc, pools, h_sb[:, i, :], xnt, ident_sb, f'a{l}_{i}')
                                psq = pqps.tile([128, 512], F32, tag="psq", bufs=2,
                                                name=f"psq_{l}_{i}")
                                pskv = pqps.tile([128, 256], F32, tag="pskv", bufs=2,
                                                 name=f"pskv_{l}_{i}")
                                for k in range(32):
                                    nc.tensor.matmul(psq[:], xnt[:, k, :], wq_sb[:, k, :],
                                                     start=(k == 0), stop=(k == 31))
                                    nc.tensor.matmul(pskv[:], xnt[:, k, :], wkv_sb[:, k, :],
                                                     start=(k == 0), stop=(k == 31))
                                q_rot = pqkv.tile([128, 512], BF16, tag="q_rot", bufs=2,
                                                  name=f"qr_{l}_{i}")
                                k_rot = pqkv.tile([128, 128], BF16, tag="k_rot", bufs=2,
                                                  name=f"kr_{l}_{i}")
                                _rope(nc, pools, psq[:], cos4_sb[:, i, :], sf4_sb[:, i, :],
                                      q_rot, 4, i)
                                _rope(nc, pools, pskv[:, 0:128], cos4_sb[:, i, 0:128],
                                      sf4_sb[:, i, 0:128], k_rot, 1, i)
                                nc.any.tensor_copy(v_sb[:, i, :], pskv[:, 128:256])
                                for hh in range(4):
                                    ptr = pqps.tile([128, 128], F32, tag="ptr", bufs=2,
                                                    name=f"ptrq_{l}_{i}_{hh}")
                                    nc.tensor.matmul(ptr[:], q_rot[:, hh * 128:(hh + 1) * 128],
                                                     ident_sb[:], start=True, stop=True)
                                    nc.any.tensor_copy(qT_sb[:, hh, i * 128:(i + 1) * 128], ptr[:])
                                ptrk = pqps.tile([128, 128], F32, tag="ptr", bufs=2,
                                                 name=f"ptrk_{l}_{i}")
                                nc.tensor.matmul(ptrk[:], k_rot[:], ident_sb[:],
                                                 start=True, stop=True)
                                nc.any.tensor_copy(kT_sb[:, i * 128:(i + 1) * 128], ptrk[:])

                        with (
                            tc.tile_pool(name="phd", bufs=1) as phd,
                            tc.tile_pool(name="phps", bufs=1, space="PSUM") as phps,
                        ):
                            for hh in range(4):
                                for i in range(8):
                                    n2 = 128 * (i + 1)
                                    pss = phps.tile([128, 1024], F32, tag="pss", bufs=2,
                                                    name=f"pss_{l}_{hh}_{i}")
                                    lhs_q = qT_sb[:, hh, i * 128:(i + 1) * 128]
                                    c0 = 0
                                    while c0 < n2 - 128:
                                        N = min(512, n2 - 128 - c0)
                                        nc.tensor.matmul(pss[:, c0:c0 + N], lhs_q,
                                                         kT_sb[:, c0:c0 + N],
                                                         start=True, stop=True)
                                        c0 += N
                                    nc.tensor.matmul(pss[:, n2 - 128:n2], lhs_q,
                                                     kT_sb[:, n2 - 128:n2],
                                                     start=True, stop=False)
                                    nc.tensor.matmul(pss[:, n2 - 128:n2], ident_sb[:],
                                                     cmask_sb[:], start=False, stop=True)
                                    mx = psmall.tile([128, 1], F32, tag="mx", bufs=2,
                                                     name=f"mx_{l}_{hh}_{i}")
                                    nc.vector.tensor_reduce(mx[:], pss[:, :n2], axis=AX.X,
                                                            op=ALU.max)
                                    negm = psmall.tile([128, 1], F32, tag="negm", bufs=2,
                                                       name=f"negm_{l}_{hh}_{i}")
                                    nc.vector.tensor_scalar_mul(negm[:], mx[:], -1.0)
                                    sume = psmall.tile([128, 1], F32, tag="sume", bufs=2,
                                                       name=f"sume_{l}_{hh}_{i}")
                                    exp_sb = phd.tile([128, 1024], BF16, tag="exp", bufs=2,
                                                      name=f"exp_{l}_{hh}_{i}")
                                    nc.scalar.activation(exp_sb[:, :n2], pss[:, :n2], AF.Exp,
                                                         bias=negm[:], accum_out=sume[:])
                                    rec = psmall.tile([128, 1], F32, tag="rec", bufs=2,
                                                      name=f"rec_{l}_{hh}_{i}")
                                    nc.vector.reciprocal(rec[:], sume[:])
                                    diag_r = phd.tile([128, 128], BF16, tag="diag_r", bufs=2,
                                                      name=f"diagr_{l}_{hh}_{i}")
                                    nc.vector.tensor_scalar_mul(diag_r[:], ident_sb[:], rec[:])
                                    atcol = phd.tile([128, 8, 128], BF16, tag="atcol", bufs=2,
                                                     name=f"atcol_{l}_{hh}_{i}")
                                    for j in range(i + 1):
                                        pat = phps.tile([128, 128], F32, tag="pat", bufs=2,
                                                        name=f"pat_{l}_{hh}_{i}_{j}")
                                        nc.tensor.matmul(pat[:], exp_sb[:, j * 128:(j + 1) * 128],
                                                         diag_r[:], start=True, stop=True)
                                        nc.any.tensor_copy(atcol[:, j, :], pat[:])
                                    pso = phps.tile([128, 128], F32, tag="pso", bufs=2,
                                                    name=f"pso_{l}_{hh}_{i}")
                                    for j in range(i + 1):
                                        nc.tensor.matmul(pso[:], v_sb[:, j, :], atcol[:, j, :],
                                                         start=(j == 0), stop=(j == i))
                                    nc.any.tensor_copy(oT_sb[:, hh, i * 128:(i + 1) * 128], pso[:])

                        with (
                            tc.tile_pool(name="pop", bufs=1) as pop,
                            tc.tile_pool(name="pops", bufs=1, space="PSUM") as pops,
                        ):
                            ow_sb = pop.tile([128, 4, H], BF16)
                            nc.sync.dma_start(
                                ow_sb[:], din[f"ow{l}"].ap().rearrange("(t p) n -> p t n", p=128))
                            for i in range(8):
                                for n in range(8):
                                    pps = pops.tile([128, 512], F32, tag="pop", bufs=4,
                                                    name=f"pop_{l}_{i}_{n}")
                                    for t in range(4):
                                        nc.tensor.matmul(pps[:], oT_sb[:, t, i * 128:(i + 1) * 128],
                                                         ow_sb[:, t, n * 512:(n + 1) * 512],
                                                         start=(t == 0), stop=(t == 3))
                                    ob = pop.tile([128, 512], BF16, tag="ob", bufs=4,
                                                  name=f"ob_{l}_{i}_{n}")
                                    nc.any.tensor_copy(ob[:], pps[:])
                                    nc.sync.dma_start(
                                        ar_in[i * 128:(i + 1) * 128, n * 512:(n + 1) * 512], ob[:])

                        nc.gpsimd.collective_compute(
                            "AllReduce", ALU.add, replica_groups=rg,
                            ins=[ar_in.opt()], outs=[ar_out.opt()])

                        with tc.tile_pool(name="pres", bufs=1) as pres:
                            for i in range(8):
                                rt = pres.tile([128, H], BF16, tag="res", bufs=2,
                                               name=f"res_{l}_{i}")
                                nc.sync.dma_start(rt[:], ar_out[i * 128:(i + 1) * 128, :])
                                nc.vector.tensor_add(h_sb[:, i, :], h_sb[:, i, :], rt[:])

                    # ============================== MLP ====================
                    with (
                        tc.tile_pool(name="pml", bufs=1) as pml,
                        tc.tile_pool(name="pdr2", bufs=1, space="DRAM") as pdr2,
                    ):
                        yt_sb = pml.tile([128, 11, S], BF16)
                        ar2_in = pdr2.tile([S, H], BF16)
                        ar2_out = pdr2.tile([S, H], BF16, addr_space="Shared")

                        for ig in range(2):
                            with (
                                tc.tile_pool(name="pgu", bufs=1) as pgu,
                                tc.tile_pool(name="pgps", bufs=1, space="PSUM") as pgps,
                            ):
                                pools = (psmall, pgu, pgps)
                                xnts = []
                                for ii in range(4):
                                    i = ig * 4 + ii
                                    xnt = pgu.tile([128, 32, 128], BF16, tag="xnt2",
                                                   bufs=4, name=f"xnt2_{l}_{i}")
                                    _norm_transpose(nc, pools, h_sb[:, i, :], xnt, ident_sb, f'a{l}_{i}')
                                    xnts.append(xnt)
                                gu = {}
                                for wname, tag in ((f"gw{l}", "g"), (f"uw{l}", "u")):
                                    outs = [pgu.tile([128, IP], BF16, tag=tag, bufs=4,
                                                     name=f"{tag}_{l}_{ig}_{ii}")
                                            for ii in range(4)]
                                    gu[tag] = outs
                                    for nb in range(3):
                                        NB = 512 if nb < 2 else IP - 1024
                                        pg = [pgps.tile([128, 512], F32, tag="pg", bufs=4,
                                                        name=f"pg_{l}_{ig}_{tag}_{nb}_{ii}")
                                              for ii in range(4)]
                                        for k in range(32):
                                            wt = pgu.tile([128, 512], BF16, tag="wstream",
                                                          bufs=4,
                                                          name=f"wt_{l}_{ig}_{tag}_{nb}_{k}")
                                            nc.sync.dma_start(
                                                wt[:, :NB],
                                                din[wname].ap()[k * 128:(k + 1) * 128,
                                                                nb * 512:nb * 512 + NB])
                                            for ii in range(4):
                                                nc.tensor.matmul(pg[ii][:, :NB],
                                                                 xnts[ii][:, k, :], wt[:, :NB],
                                                                 start=(k == 0), stop=(k == 31))
                                        for ii in range(4):
                                            nc.any.tensor_copy(
                                                outs[ii][:, nb * 512:nb * 512 + NB],
                                                pg[ii][:, :NB])
                                for ii in range(4):
                                    i = ig * 4 + ii
                                    ysil = pgu.tile([128, IP], BF16, tag="ysil", bufs=2,
                                                    name=f"ysil_{l}_{i}")
                                    nc.scalar.activation(ysil[:], gu["g"][ii][:], AF.Silu)
                                    y = pgu.tile([128, IP], BF16, tag="y", bufs=2,
                                                 name=f"y_{l}_{i}")
                                    nc.vector.tensor_mul(y[:], ysil[:], gu["u"][ii][:])
                                    for tq in range(3):
                                        ts = [tq * 4 + j for j in range(4) if tq * 4 + j < 11]
                                        ptr = pgps.tile([128, 512], F32, tag="ytr", bufs=2,
                                                        name=f"ytr_{l}_{i}_{tq}")
                                        for jj, t in enumerate(ts):
                                            nc.tensor.matmul(ptr[:, jj * 128:(jj + 1) * 128],
                                                             y[:, t * 128:(t + 1) * 128],
                                                             ident_sb[:], start=True, stop=True)
                                        nc.any.tensor_copy(
                                            yt_sb[:, ts[0]:ts[0] + len(ts),
                                                  i * 128:(i + 1) * 128],
                                            ptr[:, :len(ts) * 128].rearrange(
                                                "p (j m) -> p j m", j=len(ts)))

                        with (
                            tc.tile_pool(name="pdn", bufs=1) as pdn,
                            tc.tile_pool(name="pdps", bufs=1, space="PSUM") as pdps,
                        ):
                            for n in range(8):
                                pd = [pdps.tile([128, 512], F32, tag=f"pd{i}", bufs=1,
                                                name=f"pd_{l}_{n}_{i}")
                                      for i in range(8)]
                                for t in range(11):
                                    dwt = pdn.tile([128, 512], BF16, tag="dwstream", bufs=4,
                                                   name=f"dwt_{l}_{n}_{t}")
                                    nc.sync.dma_start(
                                        dwt[:], din[f"dw{l}"].ap()[t * 128:(t + 1) * 128,
                                                                   n * 512:(n + 1) * 512])
                                    for i in range(8):
                                        nc.tensor.matmul(pd[i][:], yt_sb[:, t, i * 128:(i + 1) * 128],
                                                         dwt[:], start=(t == 0), stop=(t == 10))
                                for i in range(8):
                                    db = pdn.tile([128, 512], BF16, tag="db", bufs=4,
                                                  name=f"db_{l}_{n}_{i}")
                                    nc.any.tensor_copy(db[:], pd[i][:])
                                    nc.sync.dma_start(
                                        ar2_in[i * 128:(i + 1) * 128, n * 512:(n + 1) * 512],
                                        db[:])

                        nc.gpsimd.collective_compute(
                            "AllReduce", ALU.add, replica_groups=rg,
                            ins=[ar2_in.opt()], outs=[ar2_out.opt()])

                        with tc.tile_pool(name="pres2", bufs=1) as pres2:
                            for i in range(8):
                                rt = pres2.tile([128, H], BF16, tag="res2", bufs=2,
                                                name=f"res2_{l}_{i}")
                                nc.sync.dma_start(rt[:], ar2_out[i * 128:(i + 1) * 128, :])
                                nc.vector.tensor_add(h_sb[:, i, :], h_sb[:, i, :], rt[:])

                # spill h to DRAM so the h pool can close before lm phase
                hdram = pdram.tile([S, H], BF16)
                for i in range(8):
                    nc.sync.dma_start(hdram[i * 128:(i + 1) * 128, :], h_sb[:, i, :])
            hstack.close()  # release h pool

            # ======================= final norm -> xf ======================
            with tc.tile_pool(name="pxf", bufs=1) as pxf:
                xf_sb = pxf.tile([128, 32, S], BF16)
                with (
                    tc.tile_pool(name="pfn", bufs=1) as pfn,
                    tc.tile_pool(name="pfps", bufs=1, space="PSUM") as pfps,
                ):
                    pools = (psmall, pfn, pfps)
                    for i in range(8):
                        ht = pfn.tile([128, H], BF16, tag="hfin", bufs=2,
                                      name=f"hfin_{i}")
                        nc.sync.dma_start(ht[:], hdram[i * 128:(i + 1) * 128, :])
                        dst = xf_sb[:, :, i * 128:(i + 1) * 128]
                        _norm_transpose(nc, pools, ht[:], dst, ident_sb, f"f{i}")
                self_lm_phases(nc, tc, psmall, xf_sb, ident_sb, ones_sb,
                               wsel_d, lmw_d, tlog_o, gmax_o, gsum_o, rg)

    nc.compile()
    return nc


def self_lm_phases(nc, tc, psmall, xf_sb, ident_sb, ones_sb, wsel_d, lmw_d,
                   tlog_o, gmax_o, gsum_o, rg):
            if True:
                pass
            with (
                tc.tile_pool(name="ptl", bufs=1) as ptl,
                tc.tile_pool(name="ptps", bufs=1, space="PSUM") as ptps,
            ):
                pt0 = ptps.tile([1, 512], F32)
                pt1 = ptps.tile([1, 512], F32)
                for k in range(32):
                    ws = ptl.tile([128, S], BF16, tag="wsel", bufs=3, name=f"ws_{k}")
                    nc.sync.dma_start(ws[:], wsel_d.ap()[k * 128:(k + 1) * 128, :])
                    tm = ptl.tile([128, S], BF16, tag="tm", bufs=2, name=f"tm_{k}")
                    nc.vector.tensor_mul(tm[:], xf_sb[:, k, :], ws[:])
                    nc.tensor.matmul(pt0[:], ones_sb[:], tm[:, :512],
                                     start=(k == 0), stop=(k == 31))
                    nc.tensor.matmul(pt1[:], ones_sb[:], tm[:, 512:],
                                     start=(k == 0), stop=(k == 31))
                tl_sb = ptl.tile([1, S], F32)
                nc.any.tensor_copy(tl_sb[:, :512], pt0[:])
                nc.any.tensor_copy(tl_sb[:, 512:], pt1[:])
                nc.sync.dma_start(tlog_o.ap(), tl_sb[:])

            with (
                tc.tile_pool(name="plm", bufs=1) as plm,
                tc.tile_pool(name="plps", bufs=1, space="PSUM") as plps,
                tc.tile_pool(name="pld", bufs=1, space="DRAM") as pld,
            ):
                logits = [plm.tile([128, VS], BF16, tag=f"lg{i}", bufs=1,
                                   name=f"logits_{i}") for i in range(8)]
                for vb in range(8):
                    pl = [plps.tile([128, 500], F32, tag=f"pl{i}", bufs=1,
                                    name=f"pl_{vb}_{i}") for i in range(8)]
                    for k in range(32):
                        lt = plm.tile([128, 500], BF16, tag="lmw", bufs=4,
                                      name=f"lt_{vb}_{k}")
                        nc.sync.dma_start(lt[:], lmw_d.ap()[vb, k * 128:(k + 1) * 128, :])
                        for i in range(8):
                            nc.tensor.matmul(pl[i][:], xf_sb[:, k, i * 128:(i + 1) * 128],
                                             lt[:], start=(k == 0), stop=(k == 31))
                    for i in range(8):
                        nc.any.tensor_copy(logits[i][:, vb * 500:(vb + 1) * 500], pl[i][:])

                gmax_sb = plm.tile([128, 8], F32)
                for i in range(8):
                    nc.vector.tensor_reduce(gmax_sb[:, i:i + 1], logits[i][:],
                                            axis=AX.X, op=ALU.max)
                gm_in = pld.tile([128, 8], F32)
                gm_out = pld.tile([128, 8], F32, addr_space="Shared")
                nc.sync.dma_start(gm_in[:], gmax_sb[:])
                nc.gpsimd.collective_compute("AllReduce", ALU.max, replica_groups=rg,
                                             ins=[gm_in.opt()], outs=[gm_out.opt()])
                gm_sb = plm.tile([128, 8], F32)
                nc.sync.dma_start(gm_sb[:], gm_out[:])
                nc.sync.dma_start(gmax_o.ap(), gm_sb[:])
                negg = plm.tile([128, 8], F32)
                nc.vector.tensor_scalar_mul(negg[:], gm_sb[:], -1.0)
                gs_sb = plm.tile([128, 8], F32)
                for i in range(8):
                    scr = plm.tile([128, VS], BF16, tag="scr", bufs=2, name=f"scr_{i}")
                    nc.scalar.activation(scr[:], logits[i][:], AF.Exp,
                                         bias=negg[:, i:i + 1],
                                         accum_out=gs_sb[:, i:i + 1])
                gs_in = pld.tile([128, 8], F32)
                gs_out = pld.tile([128, 8], F32, addr_space="Shared")
                nc.sync.dma_start(gs_in[:], gs_sb[:])
                nc.gpsimd.collective_compute("AllReduce", ALU.add, replica_groups=rg,
                                             ins=[gs_in.opt()], outs=[gs_out.opt()])
                gsf_sb = plm.tile([128, 8], F32)
                nc.sync.dma_start(gsf_sb[:], gs_out[:])
                nc.sync.dma_start(gsum_o.ap(), gsf_sb[:])


# ------------------------------------------------------------------- host --

def host_prep(inputs):
    inp = {k: np.asarray(v) for k, v in inputs.items()}
    embed = inp["embed"].astype(np.float32)
    ids = inp["input_ids"].reshape(-1).astype(np.int64)
    labels = inp["labels"].reshape(-1).astype(np.int64)

    h = embed[ids]
    cw = inp["conv_w"].astype(np.float32)
    logit = h[:-1] @ cw[0, :H] + h[1:] @ cw[0, H:] + np.float32(inp["conv_b"][0])
    mask = logit > 0
    m = np.concatenate([mask, [False]])
    hn = np.where(m[:, None], 0.5 * (h + np.roll(h, -1, axis=0)), h)
    keep = np.concatenate([[True], ~mask])
    order = np.argsort(~keep, kind="stable")
    h0 = hn[order]
    lab = labels[order]
    valid_len = int(keep.sum())

    inv = 1.0 / (THETA ** (np.arange(0, HD, 2, dtype=np.float32) / HD))
    t = np.arange(S, dtype=np.float32)
    freqs = np.outer(t, inv)
    emb = np.concatenate([freqs, freqs], -1)
    cos, sin = np.cos(emb), np.sin(emb)
    sinflip = np.concatenate([-sin[:, :HD // 2], sin[:, HD // 2:]], -1)
    cos4 = np.tile(cos, (1, 4)).astype(bf16)
    sinflip4 = np.tile(sinflip, (1, 4)).astype(bf16)

    ident = np.eye(128, dtype=bf16)
    cmask = np.where(np.arange(128)[None, :] > np.arange(128)[:, None],
                     np.float32(NEG), np.float32(0)).astype(bf16)
    ones = np.ones((128, 1), dtype=bf16)

    ln1 = inp["ln1_w"].astype(np.float32)
    ln2 = inp["ln2_w"].astype(np.float32)
    normw = inp["norm_w"].astype(np.float32)
    qsc = np.float32(1.0 / np.sqrt(HD))
    lm_folded = normw[:, None] * inp["lm_head_w"].astype(np.float32)
    tgt = np.concatenate([lab[1:], [0]]).astype(np.int64)
    wsel = np.ascontiguousarray(lm_folded[:, tgt]).astype(bf16)

    common = dict(h0=h0.astype(bf16), cos4=cos4, sf4=sinflip4, ident=ident,
                  cmask=cmask, ones=ones, wsel=wsel)
    in_maps = []
    for c in range(NC_):
        mcore = dict(common)
        for l in range(L):
            qw = ln1[l][:, None] * inp["q_w"][l].astype(np.float32) * qsc
            kw = ln1[l][:, None] * inp["k_w"][l].astype(np.float32)
            vw = ln1[l][:, None] * inp["v_w"][l].astype(np.float32)
            gw = ln2[l][:, None] * inp["gate_w"][l].astype(np.float32)
            uw = ln2[l][:, None] * inp["up_w"][l].astype(np.float32)
            dw = inp["down_w"][l].astype(np.float32)
            gws = np.zeros((H, IP), np.float32)
            uws = np.zeros((H, IP), np.float32)
            dws = np.zeros((IP, H), np.float32)
            gws[:, :IPC] = gw[:, c * IPC:(c + 1) * IPC]
            uws[:, :IPC] = uw[:, c * IPC:(c + 1) * IPC]
            dws[:IPC] = dw[c * IPC:(c + 1) * IPC]
            mcore[f"qw{l}"] = np.ascontiguousarray(qw[:, c * 512:(c + 1) * 512]).astype(bf16)
            mcore[f"kvw{l}"] = np.concatenate(
                [kw[:, c * 128:(c + 1) * 128], vw[:, c * 128:(c + 1) * 128]],
                1).astype(bf16)
            mcore[f"ow{l}"] = np.ascontiguousarray(
                inp["o_w"][l][c * 512:(c + 1) * 512].astype(np.float32)).astype(bf16)
            mcore[f"gw{l}"] = gws.astype(bf16)
            mcore[f"uw{l}"] = uws.astype(bf16)
            mcore[f"dw{l}"] = dws.astype(bf16)
        lmc = lm_folded[:, c * VS:(c + 1) * VS]
        mcore["lmw"] = np.ascontiguousarray(
            lmc.reshape(H, 8, VS // 8).transpose(1, 0, 2)).astype(bf16)
        in_maps.append(mcore)

    return in_maps, valid_len


def kernel(**inputs) -> np.ndarray:
    in_maps, valid_len = host_prep(inputs)
    if "nc" not in _cache:
        _cache["nc"] = build_nc()
    nc = _cache["nc"]
    res = run_bass_kernel_spmd(nc, in_maps, list(range(NC_)),
                               **last_run_info.get("run_kwargs", {}))
    last_run_info["res"] = res
    out = res.results[0]
    gmax = out["gmax_o"].transpose(1, 0).reshape(S).astype(np.float64)
    gsum = out["gsum_o"].transpose(1, 0).reshape(S).astype(np.float64)
    tlog = out["tlog_o"].reshape(S).astype(np.float64)
    ce = gmax + np.log(gsum) - tlog
    w = (np.arange(S - 1) < valid_len - 1).astype(np.float64)
    loss = (ce[:S - 1] * w).sum() / w.sum()
    return np.float32(loss)


# revision 12
# speedup vs baseline: 1.0423x; 1.0423x over previous
"""Trainium2 Bass kernel for nn_Decoder_20486994002617.

8-core tensor-parallel 2-layer llama-style decoder with ragged token-merge
(handled on host), returning the masked-mean cross-entropy loss.

Device layout choices:
  - h (residual) lives in SBUF as [128 part, 8 seq-tiles, 4096] bf16.
  - RMSNorm weights are folded into the consumer weight matrices on host,
    so the device norm is x * rsqrt(mean(x^2)+eps) only; the multiply by
    the per-row factor is fused into the seq->feature transpose as a
    matmul against diag(factor).
  - Attention: heads sharded 4 q-heads + 1 kv-head per core (GQA groups
    align), scores/softmax per (head, 128-row tile), causal mask added via
    an extra accumulating matmul (I.T @ cmask), attn probs transposed back
    through the PE with diag(1/sumexp) fused.
  - MLP: intermediate dim sharded 1376/core, padded to 1408 = 11*128.
  - lm_head: vocab sharded 4000/core; softmax stats (row max, sum-exp) are
    AllReduce'd; the target logit is computed via a host-gathered column
    matrix (wsel) so no device gather is needed.
Outputs per core: gmax [128,8] f32, gsum [128,8] f32, tlog [1,1024] f32.
Host finishes: ce = gmax + log(gsum) - tlog; loss = masked mean.
"""
import numpy as np
import ml_dtypes

from contextlib import ExitStack

import concourse.bass as bass
import concourse.bacc as bacc
import concourse.mybir as mybir
import concourse.tile as tile
from concourse.bass_utils import run_bass_kernel_spmd

F32 = mybir.dt.float32
BF16 = mybir.dt.bfloat16
AF = mybir.ActivationFunctionType
ALU = mybir.AluOpType
AX = mybir.AxisListType

H, HD, NH, NKV = 4096, 128, 32, 8
L, V, S, I = 2, 32000, 1024, 11008
EPS, THETA = 1e-6, 10000.0
NC_ = 8          # cores
IPC = I // NC_   # 1376
IP = 1408        # padded intermediate per core = 11 * 128
VS = V // NC_    # 4000 vocab per core
NEG = -1e9

bf16 = ml_dtypes.bfloat16

last_run_info = {}
_cache = {}


# ----------------------------------------------------------------- device --

def _norm_transpose(nc, pools, h_ap, dst, ident_sb, uid):
    """dst[:, k, :] (32 chunks of [128,128]) = normalized transpose of
    h_ap ([128 seq rows, 4096]). dst free dims must be (32, 128)."""
    small, ntmp, psum = pools
    sq = ntmp.tile([128, 4096], BF16, tag="nt_sq", bufs=1, name=f"sq_{uid}")
    ssq = small.tile([128, 1], F32, tag="nt_ssq", bufs=2, name=f"ssq_{uid}")
    nc.scalar.activation(sq[:], h_ap, AF.Square, accum_out=ssq[:])
    var = small.tile([128, 1], F32, tag="nt_var", bufs=2, name=f"var_{uid}")
    nc.vector.tensor_scalar(var[:], ssq[:], 1.0 / H, EPS, op0=ALU.mult, op1=ALU.add)
    std = small.tile([128, 1], F32, tag="nt_std", bufs=2, name=f"std_{uid}")
    nc.scalar.sqrt(std[:], var[:])
    fac = small.tile([128, 1], F32, tag="nt_fac", bufs=2, name=f"fac_{uid}")
    nc.vector.reciprocal(fac[:], std[:])
    diag = ntmp.tile([128, 128], BF16, tag="nt_diag", bufs=2, name=f"diag_{uid}")
    nc.vector.tensor_scalar_mul(diag[:], ident_sb[:], fac[:])
    for kk in range(8):
        pnt = psum.tile([128, 512], F32, tag="nt_ps", bufs=2, name=f"pnt_{uid}_{kk}")
        for j in range(4):
            k = kk * 4 + j
            nc.tensor.matmul(pnt[:, j * 128:(j + 1) * 128],
                             h_ap[:, k * 128:(k + 1) * 128], diag[:],
                             start=True, stop=True)
        nc.any.tensor_copy(dst[:, kk * 4:(kk + 1) * 4, :],
                           pnt[:].rearrange("p (j m) -> p j m", j=4))


def _rope(nc, pools, ps, cos_ap, sf_ap, out, nheads, i):
    """out (bf16 [128, nheads*128]) = rope(ps) with ps a psum slice."""
    small, ntmp, psum = pools
    n = nheads * 128
    t1 = ntmp.tile([128, 512], F32, tag="rope_t1", bufs=2, name=f"t1_{i}_{nheads}")
    t2 = ntmp.tile([128, 512], F32, tag="rope_t2", bufs=2, name=f"t2_{i}_{nheads}")
    nc.vector.tensor_mul(t1[:, :n], ps, cos_ap)
    for hh in range(nheads):
        b = hh * 128
        nc.vector.tensor_mul(t2[:, b:b + 64], ps[:, b + 64:b + 128],
                             sf_ap[:, b:b + 64])
        nc.vector.tensor_mul(t2[:, b + 64:b + 128], ps[:, b:b + 64],
                             sf_ap[:, b + 64:b + 128])
    nc.vector.tensor_add(out[:], t1[:, :n], t2[:, :n])


def build_nc():
    nc = bacc.Bacc("TRN2", target_bir_lowering=False, debug=False,
                   num_devices=NC_)

    din = {}
    def dram_in(name, shape):
        din[name] = nc.dram_tensor(name, shape, BF16, kind="ExternalInput")
        return din[name]

    h0_d = dram_in("h0", [S, H])
    cos4_d = dram_in("cos4", [S, 512])
    sf4_d = dram_in("sf4", [S, 512])
    ident_d = dram_in("ident", [128, 128])
    cmask_d = dram_in("cmask", [128, 128])
    ones_d = dram_in("ones", [128, 1])
    for l in range(L):
        dram_in(f"qw{l}", [H, 512])
        dram_in(f"kvw{l}", [H, 256])
        dram_in(f"ow{l}", [512, H])
        dram_in(f"gw{l}", [H, IP])
        dram_in(f"uw{l}", [H, IP])
        dram_in(f"dw{l}", [IP, H])
    lmw_d = dram_in("lmw", [8, H, VS // 8])
    wsel_d = dram_in("wsel", [H, S])

    gmax_o = nc.dram_tensor("gmax_o", [128, 8], F32, kind="ExternalOutput")
    gsum_o = nc.dram_tensor("gsum_o", [128, 8], F32, kind="ExternalOutput")
    tlog_o = nc.dram_tensor("tlog_o", [1, S], F32, kind="ExternalOutput")

    rg = [list(range(NC_))]

    with tile.TileContext(nc) as tc:
        with (
            tc.tile_pool(name="pconst", bufs=1) as pconst,
            tc.tile_pool(name="psmall", bufs=1) as psmall,
            tc.tile_pool(name="pdram", bufs=1, space="DRAM") as pdram,
        ):
            ident_sb = pconst.tile([128, 128], BF16)
            cmask_sb = pconst.tile([128, 128], BF16)
            ones_sb = pconst.tile([128, 1], BF16)
            cos4_sb = pconst.tile([128, 8, 512], BF16)
            sf4_sb = pconst.tile([128, 8, 512], BF16)
            nc.sync.dma_start(ident_sb[:], ident_d.ap())
            nc.sync.dma_start(cmask_sb[:], cmask_d.ap())
            nc.sync.dma_start(ones_sb[:], ones_d.ap())
            for i in range(8):
                nc.sync.dma_start(cos4_sb[:, i, :], cos4_d.ap()[i * 128:(i + 1) * 128, :])
                nc.sync.dma_start(sf4_sb[:, i, :], sf4_d.ap()[i * 128:(i + 1) * 128, :])

            hstack = ExitStack()
            phh = hstack.enter_context(tc.tile_pool(name="phh", bufs=1))
            if True:
                h_sb = phh.tile([128, 8, H], BF16)
                for i in range(8):
                    nc.sync.dma_start(h_sb[:, i, :], h0_d.ap()[i * 128:(i + 1) * 128, :])

                for l in range(L):
                    # ============================== attention ==============
                    with (
                        tc.tile_pool(name="pal", bufs=1) as pal,      # attn-long
                        tc.tile_pool(name="pdr", bufs=1, space="DRAM") as pdr,
                    ):
                        qT_sb = pal.tile([128, 4, S], BF16)
                        kT_sb = pal.tile([128, S], BF16)
                        v_sb = pal.tile([128, 8, 128], BF16)
                        oT_sb = pal.tile([128, 4, S], BF16)
                        ar_in = pdr.tile([S, H], BF16)
                        ar_outs = [pdr.tile([512, H], BF16, addr_space="Shared",
                                            name=f"ar_out_{l}_{c}") for c in range(2)]

                        with (
                            tc.tile_pool(name="pqkv", bufs=1) as pqkv,
                            tc.tile_pool(name="pqps", bufs=1, space="PSUM") as pqps,
                        ):
                            pools = (psmall, pqkv, pqps)
                            wq_sb = pqkv.tile([128, 32, 512], BF16)
                            wkv_sb = pqkv.tile([128, 32, 256], BF16)
                            nc.sync.dma_start(
                                wq_sb[:], din[f"qw{l}"].ap().rearrange("(k p) n -> p k n", p=128))
                            nc.sync.dma_start(
                                wkv_sb[:], din[f"kvw{l}"].ap().rearrange("(k p) n -> p k n", p=128))
                            for i in range(8):
                                xnt = pqkv.tile([128, 32, 128], BF16, tag="xnt",
                                                bufs=2, name=f"xnt_{l}_{i}")
                                _norm_transpose(nc, pools, h_sb[:, i, :], xnt, ident_sb, f'a{l}_{i}')
                                psq = pqps.tile([128, 512], F32, tag="psq", bufs=2,
                                                name=f"psq_{l}_{i}")
                                pskv = pqps.tile([128, 256], F32, tag="pskv", bufs=2,
                                                 name=f"pskv_{l}_{i}")
                                for k in range(32):
                                    nc.tensor.matmul(psq[:], xnt[:, k, :], wq_sb[:, k, :],
                                                     start=(k == 0), stop=(k == 31))
                                    nc.tensor.matmul(pskv[:], xnt[:, k, :], wkv_sb[:, k, :],
                                                     start=(k == 0), stop=(k == 31))
                                q_rot = pqkv.tile([128, 512], BF16, tag="q_rot", bufs=2,
                                                  name=f"qr_{l}_{i}")
                                k_rot = pqkv.tile([128, 128], BF16, tag="k_rot", bufs=2,
                                                  name=f"kr_{l}_{i}")
                                _rope(nc, pools, psq[:], cos4_sb[:, i, :], sf4_sb[:, i, :],
                                      q_rot, 4, i)
                                _rope(nc, pools, pskv[:, 0:128], cos4_sb[:, i, 0:128],
                                      sf4_sb[:, i, 0:128], k_rot, 1, i)
                                nc.any.tensor_copy(v_sb[:, i, :], pskv[:, 128:256])
                                for hh in range(4):
                                    ptr = pqps.tile([128, 128], F32, tag="ptr", bufs=2,
                                                    name=f"ptrq_{l}_{i}_{hh}")
                                    nc.tensor.matmul(ptr[:], q_rot[:, hh * 128:(hh + 1) * 128],
                                                     ident_sb[:], start=True, stop=True)
                                    nc.any.tensor_copy(qT_sb[:, hh, i * 128:(i + 1) * 128], ptr[:])
                                ptrk = pqps.tile([128, 128], F32, tag="ptr", bufs=2,
                                                 name=f"ptrk_{l}_{i}")
                                nc.tensor.matmul(ptrk[:], k_rot[:], ident_sb[:],
                                                 start=True, stop=True)
                                nc.any.tensor_copy(kT_sb[:, i * 128:(i + 1) * 128], ptrk[:])

                        with (
                            tc.tile_pool(name="phd", bufs=1) as phd,
                            tc.tile_pool(name="phps", bufs=1, space="PSUM") as phps,
                        ):
                            for hh in range(4):
                                for i in range(8):
                                    n2 = 128 * (i + 1)
                                    pss = phps.tile([128, 1024], F32, tag="pss", bufs=2,
                                                    name=f"pss_{l}_{hh}_{i}")
                                    lhs_q = qT_sb[:, hh, i * 128:(i + 1) * 128]
                                    c0 = 0
                                    while c0 < n2 - 128:
                                        N = min(512, n2 - 128 - c0)
                                        nc.tensor.matmul(pss[:, c0:c0 + N], lhs_q,
                                                         kT_sb[:, c0:c0 + N],
                                                         start=True, stop=True)
                                        c0 += N
                                    nc.tensor.matmul(pss[:, n2 - 128:n2], lhs_q,
                                                     kT_sb[:, n2 - 128:n2],
                                                     start=True, stop=False)
                                    nc.tensor.matmul(pss[:, n2 - 128:n2], ident_sb[:],
                                                     cmask_sb[:], start=False, stop=True)
                                    mx = psmall.tile([128, 1], F32, tag="mx", bufs=2,
                                                     name=f"mx_{l}_{hh}_{i}")
                                    nc.vector.tensor_reduce(mx[:], pss[:, :n2], axis=AX.X,
                                                            op=ALU.max)
                                    negm = psmall.tile([128, 1], F32, tag="negm", bufs=2,
                                                       name=f"negm_{l}_{hh}_{i}")
                                    nc.vector.tensor_scalar_mul(negm[:], mx[:], -1.0)
                                    sume = psmall.tile([128, 1], F32, tag="sume", bufs=2,
                                                       name=f"sume_{l}_{hh}_{i}")
                                    exp_sb = phd.tile([128, 1024], BF16, tag="exp", bufs=2,
                                                      name=f"exp_{l}_{hh}_{i}")
                                    nc.scalar.activation(exp_sb[:, :n2], pss[:, :n2], AF.Exp,
                                                         bias=negm[:], accum_out=sume[:])
                                    rec = psmall.tile([128, 1], F32, tag="rec", bufs=2,
                                                      name=f"rec_{l}_{hh}_{i}")
                                    nc.vector.reciprocal(rec[:], sume[:])
                                    diag_r = phd.tile([128, 128], BF16, tag="diag_r", bufs=2,
                                                      name=f"diagr_{l}_{hh}_{i}")
                                    nc.vector.tensor_scalar_mul(diag_r[:], ident_sb[:], rec[:])
                                    atcol = phd.tile([128, 8, 128], BF16, tag="atcol", bufs=2,
                                                     name=f"atcol_{l}_{hh}_{i}")
                                    for j in range(i + 1):
                                        pat = phps.tile([128, 128], F32, tag="pat", bufs=2,
                                                        name=f"pat_{l}_{hh}_{i}_{j}")
                                        nc.tensor.matmul(pat[:], exp_sb[:, j * 128:(j + 1) * 128],
                                                         diag_r[:], start=True, stop=True)
                                        nc.any.tensor_copy(atcol[:, j, :], pat[:])
                                    pso = phps.tile([128, 128], F32, tag="pso", bufs=2,
                                                    name=f"pso_{l}_{hh}_{i}")
                                    for j in range(i + 1):
                                        nc.tensor.matmul(pso[:], v_sb[:, j, :], atcol[:, j, :],
                                                         start=(j == 0), stop=(j == i))
                                    nc.any.tensor_copy(oT_sb[:, hh, i * 128:(i + 1) * 128], pso[:])

                        with (
                            tc.tile_pool(name="pop", bufs=1) as pop,
                            tc.tile_pool(name="pops", bufs=1, space="PSUM") as pops,
                        ):
                            ow_sb = pop.tile([128, 4, H], BF16)
                            nc.sync.dma_start(
                                ow_sb[:], din[f"ow{l}"].ap().rearrange("(t p) n -> p t n", p=128))
                            for i in range(8):
                                ob = pop.tile([128, H], BF16, tag="ob", bufs=3,
                                              name=f"ob_{l}_{i}")
                                for n in range(8):
                                    pps = pops.tile([128, 512], F32, tag="pop", bufs=4,
                                                    name=f"pop_{l}_{i}_{n}")
                                    for t in range(4):
                                        nc.tensor.matmul(pps[:], oT_sb[:, t, i * 128:(i + 1) * 128],
                                                         ow_sb[:, t, n * 512:(n + 1) * 512],
                                                         start=(t == 0), stop=(t == 3))
                                    nc.any.tensor_copy(ob[:, n * 512:(n + 1) * 512], pps[:])
                                nc.sync.dma_start(
                                    ar_in[i * 128:(i + 1) * 128, :], ob[:])
                                if i == 3:
                                    nc.gpsimd.collective_compute(
                                        "AllReduce", ALU.add, replica_groups=rg,
                                        ins=[ar_in[0:512, :].opt()], outs=[ar_outs[0].opt()])
                            nc.gpsimd.collective_compute(
                                "AllReduce", ALU.add, replica_groups=rg,
                                ins=[ar_in[512:1024, :].opt()], outs=[ar_outs[1].opt()])

                        with tc.tile_pool(name="pres", bufs=1) as pres:
                            for half in range(2):
                                rt = pres.tile([128, 4, H], BF16, tag="res", bufs=2,
                                               name=f"res_{l}_{half}")
                                nc.sync.dma_start(
                                    rt[:], ar_outs[half][:]
                                    .rearrange("(i p) n -> p i n", p=128))
                                for ii in range(4):
                                    i = half * 4 + ii
                                    nc.vector.tensor_add(h_sb[:, i, :], h_sb[:, i, :],
                                                         rt[:, ii, :])

                    # ============================== MLP ====================
                    with (
                        tc.tile_pool(name="pml", bufs=1) as pml,
                        tc.tile_pool(name="pdr2", bufs=1, space="DRAM") as pdr2,
                    ):
                        yt_sb = pml.tile([128, 11, S], BF16)
                        ar2_in = pdr2.tile([S, H], BF16)
                        ar2_outs = [pdr2.tile([512, H], BF16, addr_space="Shared",
                                              name=f"ar2_out_{l}_{c}") for c in range(2)]

                        for ig in range(2):
                            with (
                                tc.tile_pool(name="pgu", bufs=1) as pgu,
                                tc.tile_pool(name="pgps", bufs=1, space="PSUM") as pgps,
                            ):
                                pools = (psmall, pgu, pgps)
                                xnts = []
                                for ii in range(4):
                                    i = ig * 4 + ii
                                    xnt = pgu.tile([128, 32, 128], BF16, tag="xnt2",
                                                   bufs=4, name=f"xnt2_{l}_{i}")
                                    _norm_transpose(nc, pools, h_sb[:, i, :], xnt, ident_sb, f'a{l}_{i}')
                                    xnts.append(xnt)
                                gu = {}
                                for wname, tag in ((f"gw{l}", "g"), (f"uw{l}", "u")):
                                    outs = [pgu.tile([128, IP], BF16, tag=tag, bufs=4,
                                                     name=f"{tag}_{l}_{ig}_{ii}")
                                            for ii in range(4)]
                                    gu[tag] = outs
                                    for nb in range(3):
                                        NB = 512 if nb < 2 else IP - 1024
                                        pg = [pgps.tile([128, 512], F32, tag="pg", bufs=4,
                                                        name=f"pg_{l}_{ig}_{tag}_{nb}_{ii}")
                                              for ii in range(4)]
                                        for kp in range(8):
                                            wt = pgu.tile([128, 4, 512], BF16, tag="wstream",
                                                          bufs=4,
                                                          name=f"wt_{l}_{ig}_{tag}_{nb}_{kp}")
                                            nc.sync.dma_start(
                                                wt[:, :, :NB],
                                                din[wname].ap()[kp * 512:(kp + 1) * 512,
                                                                nb * 512:nb * 512 + NB]
                                                .rearrange("(j p) n -> p j n", p=128))
                                            for jk in range(4):
                                                k = kp * 4 + jk
                                                for ii in range(4):
                                                    nc.tensor.matmul(pg[ii][:, :NB],
                                                                     xnts[ii][:, k, :],
                                                                     wt[:, jk, :NB],
                                                                     start=(k == 0), stop=(k == 31))
                                        for ii in range(4):
                                            nc.any.tensor_copy(
                                                outs[ii][:, nb * 512:nb * 512 + NB],
                                                pg[ii][:, :NB])
                                for ii in range(4):
                                    i = ig * 4 + ii
                                    ysil = pgu.tile([128, IP], BF16, tag="ysil", bufs=2,
                                                    name=f"ysil_{l}_{i}")
                                    nc.scalar.activation(ysil[:], gu["g"][ii][:], AF.Silu)
                                    y = pgu.tile([128, IP], BF16, tag="y", bufs=2,
                                                 name=f"y_{l}_{i}")
                                    nc.vector.tensor_mul(y[:], ysil[:], gu["u"][ii][:])
                                    for tq in range(3):
                                        ts = [tq * 4 + j for j in range(4) if tq * 4 + j < 11]
                                        ptr = pgps.tile([128, 512], F32, tag="ytr", bufs=2,
                                                        name=f"ytr_{l}_{i}_{tq}")
                                        for jj, t in enumerate(ts):
                                            nc.tensor.matmul(ptr[:, jj * 128:(jj + 1) * 128],
                                                             y[:, t * 128:(t + 1) * 128],
                                                             ident_sb[:], start=True, stop=True)
                                        nc.any.tensor_copy(
                                            yt_sb[:, ts[0]:ts[0] + len(ts),
                                                  i * 128:(i + 1) * 128],
                                            ptr[:, :len(ts) * 128].rearrange(
                                                "p (j m) -> p j m", j=len(ts)))

                        with (
                            tc.tile_pool(name="pdn", bufs=1) as pdn,
                            tc.tile_pool(name="pdps", bufs=1, space="PSUM") as pdps,
                        ):
                            for half in range(2):
                                dbs = [pdn.tile([128, H], BF16, tag=f"db{ii}", bufs=2,
                                                name=f"db_{l}_{half}_{ii}")
                                       for ii in range(4)]
                                for n in range(8):
                                    pd = [pdps.tile([128, 512], F32, tag=f"pd{ii}", bufs=2,
                                                    name=f"pd_{l}_{half}_{n}_{ii}")
                                          for ii in range(4)]
                                    for tp in range(3):
                                        nt = 4 if tp < 2 else 3
                                        dwt = pdn.tile([128, 4, 512], BF16, tag="dwstream",
                                                       bufs=4, name=f"dwt_{l}_{half}_{n}_{tp}")
                                        nc.sync.dma_start(
                                            dwt[:, :nt, :],
                                            din[f"dw{l}"].ap()[tp * 512:tp * 512 + nt * 128,
                                                               n * 512:(n + 1) * 512]
                                            .rearrange("(j p) n -> p j n", p=128))
                                        for jt in range(nt):
                                            t = tp * 4 + jt
                                            for ii in range(4):
                                                i = half * 4 + ii
                                                nc.tensor.matmul(
                                                    pd[ii][:], yt_sb[:, t, i * 128:(i + 1) * 128],
                                                    dwt[:, jt, :], start=(t == 0), stop=(t == 10))
                                    for ii in range(4):
                                        nc.any.tensor_copy(dbs[ii][:, n * 512:(n + 1) * 512],
                                                           pd[ii][:])
                                for ii in range(4):
                                    i = half * 4 + ii
                                    nc.sync.dma_start(ar2_in[i * 128:(i + 1) * 128, :],
                                                      dbs[ii][:])
                                nc.gpsimd.collective_compute(
                                    "AllReduce", ALU.add, replica_groups=rg,
                                    ins=[ar2_in[half * 512:(half + 1) * 512, :].opt()],
                                    outs=[ar2_outs[half].opt()])

                        with tc.tile_pool(name="pres2", bufs=1) as pres2:
                            for half in range(2):
                                rt = pres2.tile([128, 4, H], BF16, tag="res2", bufs=2,
                                                name=f"res2_{l}_{half}")
                                nc.sync.dma_start(
                                    rt[:], ar2_outs[half][:]
                                    .rearrange("(i p) n -> p i n", p=128))
                                for ii in range(4):
                                    i = half * 4 + ii
                                    nc.vector.tensor_add(h_sb[:, i, :], h_sb[:, i, :],
                                                         rt[:, ii, :])

                # spill h to DRAM so the h pool can close before lm phase
                hdram = pdram.tile([S, H], BF16)
                for i in range(8):
                    nc.sync.dma_start(hdram[i * 128:(i + 1) * 128, :], h_sb[:, i, :])
            hstack.close()  # release h pool

            # ======================= final norm -> xf ======================
            with tc.tile_pool(name="pxf", bufs=1) as pxf:
                xf_sb = pxf.tile([128, 32, S], BF16)
                with (
                    tc.tile_pool(name="pfn", bufs=1) as pfn,
                    tc.tile_pool(name="pfps", bufs=1, space="PSUM") as pfps,
                ):
                    pools = (psmall, pfn, pfps)
                    for i in range(8):
                        ht = pfn.tile([128, H], BF16, tag="hfin", bufs=2,
                                      name=f"hfin_{i}")
                        nc.sync.dma_start(ht[:], hdram[i * 128:(i + 1) * 128, :])
                        dst = xf_sb[:, :, i * 128:(i + 1) * 128]
                        _norm_transpose(nc, pools, ht[:], dst, ident_sb, f"f{i}")
                self_lm_phases(nc, tc, psmall, xf_sb, ident_sb, ones_sb,
                               wsel_d, lmw_d, tlog_o, gmax_o, gsum_o, rg)

    nc.compile()
    return nc


def self_lm_phases(nc, tc, psmall, xf_sb, ident_sb, ones_sb, wsel_d, lmw_d,
                   tlog_o, gmax_o, gsum_o, rg):
            if True:
                pass
            with (
                tc.tile_pool(name="ptl", bufs=1) as ptl,
                tc.tile_pool(name="ptps", bufs=1, space="PSUM") as ptps,
            ):
                pt0 = ptps.tile([1, 512], F32)
                pt1 = ptps.tile([1, 512], F32)
                for kp in range(8):
                    ws = ptl.tile([128, 4, S], BF16, tag="wsel", bufs=2, name=f"ws_{kp}")
                    nc.sync.dma_start(
                        ws[:], wsel_d.ap()[kp * 512:(kp + 1) * 512, :]
                        .rearrange("(j p) n -> p j n", p=128))
                    for jk in range(4):
                        k = kp * 4 + jk
                        tm = ptl.tile([128, S], BF16, tag="tm", bufs=2, name=f"tm_{k}")
                        nc.vector.tensor_mul(tm[:], xf_sb[:, k, :], ws[:, jk, :])
                        nc.tensor.matmul(pt0[:], ones_sb[:], tm[:, :512],
                                         start=(k == 0), stop=(k == 31))
                        nc.tensor.matmul(pt1[:], ones_sb[:], tm[:, 512:],
                                         start=(k == 0), stop=(k == 31))
                tl_sb = ptl.tile([1, S], F32)
                nc.any.tensor_copy(tl_sb[:, :512], pt0[:])
                nc.any.tensor_copy(tl_sb[:, 512:], pt1[:])
                nc.sync.dma_start(tlog_o.ap(), tl_sb[:])

            with (
                tc.tile_pool(name="plm", bufs=1) as plm,
                tc.tile_pool(name="plps", bufs=1, space="PSUM") as plps,
                tc.tile_pool(name="pld", bufs=1, space="DRAM") as pld,
            ):
                logits = [plm.tile([128, VS], BF16, tag=f"lg{i}", bufs=1,
                                   name=f"logits_{i}") for i in range(8)]
                for vb in range(8):
                    pl = [plps.tile([128, 500], F32, tag=f"pl{i}", bufs=1,
                                    name=f"pl_{vb}_{i}") for i in range(8)]
                    for kp in range(8):
                        lt = plm.tile([128, 4, 500], BF16, tag="lmw", bufs=4,
                                      name=f"lt_{vb}_{kp}")
                        nc.sync.dma_start(
                            lt[:], lmw_d.ap()[vb, kp * 512:(kp + 1) * 512, :]
                            .rearrange("(j p) n -> p j n", p=128))
                        for jk in range(4):
                            k = kp * 4 + jk
                            for i in range(8):
                                nc.tensor.matmul(pl[i][:], xf_sb[:, k, i * 128:(i + 1) * 128],
                                                 lt[:, jk, :], start=(k == 0), stop=(k == 31))
                    for i in range(8):
                        nc.any.tensor_copy(logits[i][:, vb * 500:(vb + 1) * 500], pl[i][:])

                gmax_sb = plm.tile([128, 8], F32)
                for i in range(8):
                    nc.vector.tensor_reduce(gmax_sb[:, i:i + 1], logits[i][:],
                                            axis=AX.X, op=ALU.max)
                gm_in = pld.tile([128, 8], F32)
                gm_out = pld.tile([128, 8], F32, addr_space="Shared")
                nc.sync.dma_start(gm_in[:], gmax_sb[:])
                nc.gpsimd.collective_compute("AllReduce", ALU.max, replica_groups=rg,
                                             ins=[gm_in.opt()], outs=[gm_out.opt()])
                gm_sb = plm.tile([128, 8], F32)
                nc.sync.dma_start(gm_sb[:], gm_out[:])
                nc.sync.dma_start(gmax_o.ap(), gm_sb[:])
                negg = plm.tile([128, 8], F32)
                nc.vector.tensor_scalar_mul(negg[:], gm_sb[:], -1.0)
                gs_sb = plm.tile([128, 8], F32)
                for i in range(8):
                    scr = plm.tile([128, VS], BF16, tag="scr", bufs=2, name=f"scr_{i}")
                    nc.scalar.activation(scr[:], logits[i][:], AF.Exp,
                                         bias=negg[:, i:i + 1],
                                         accum_out=gs_sb[:, i:i + 1])
                gs_in = pld.tile([128, 8], F32)
                gs_out = pld.tile([128, 8], F32, addr_space="Shared")
                nc.sync.dma_start(gs_in[:], gs_sb[:])
                nc.gpsimd.collective_compute("AllReduce", ALU.add, replica_groups=rg,
                                             ins=[gs_in.opt()], outs=[gs_out.opt()])
                gsf_sb = plm.tile([128, 8], F32)
                nc.sync.dma_start(gsf_sb[:], gs_out[:])
                nc.sync.dma_start(gsum_o.ap(), gsf_sb[:])


# ------------------------------------------------------------------- host --

def host_prep(inputs):
    inp = {k: np.asarray(v) for k, v in inputs.items()}
    embed = inp["embed"].astype(np.float32)
    ids = inp["input_ids"].reshape(-1).astype(np.int64)
    labels = inp["labels"].reshape(-1).astype(np.int64)

    h = embed[ids]
    cw = inp["conv_w"].astype(np.float32)
    logit = h[:-1] @ cw[0, :H] + h[1:] @ cw[0, H:] + np.float32(inp["conv_b"][0])
    mask = logit > 0
    m = np.concatenate([mask, [False]])
    hn = np.where(m[:, None], 0.5 * (h + np.roll(h, -1, axis=0)), h)
    keep = np.concatenate([[True], ~mask])
    order = np.argsort(~keep, kind="stable")
    h0 = hn[order]
    lab = labels[order]
    valid_len = int(keep.sum())

    inv = 1.0 / (THETA ** (np.arange(0, HD, 2, dtype=np.float32) / HD))
    t = np.arange(S, dtype=np.float32)
    freqs = np.outer(t, inv)
    emb = np.concatenate([freqs, freqs], -1)
    cos, sin = np.cos(emb), np.sin(emb)
    sinflip = np.concatenate([-sin[:, :HD // 2], sin[:, HD // 2:]], -1)
    cos4 = np.tile(cos, (1, 4)).astype(bf16)
    sinflip4 = np.tile(sinflip, (1, 4)).astype(bf16)

    ident = np.eye(128, dtype=bf16)
    cmask = np.where(np.arange(128)[None, :] > np.arange(128)[:, None],
                     np.float32(NEG), np.float32(0)).astype(bf16)
    ones = np.ones((128, 1), dtype=bf16)

    ln1 = inp["ln1_w"].astype(np.float32)
    ln2 = inp["ln2_w"].astype(np.float32)
    normw = inp["norm_w"].astype(np.float32)
    qsc = np.float32(1.0 / np.sqrt(HD))
    lm_folded = normw[:, None] * inp["lm_head_w"].astype(np.float32)
    tgt = np.concatenate([lab[1:], [0]]).astype(np.int64)
    wsel = np.ascontiguousarray(lm_folded[:, tgt]).astype(bf16)

    common = dict(h0=h0.astype(bf16), cos4=cos4, sf4=sinflip4, ident=ident,
                  cmask=cmask, ones=ones, wsel=wsel)
    in_maps = []
    for c in range(NC_):
        mcore = dict(common)
        for l in range(L):
            qw = ln1[l][:, None] * inp["q_w"][l].astype(np.float32) * qsc
            kw = ln1[l][:, None] * inp["k_w"][l].astype(np.float32)
            vw = ln1[l][:, None] * inp["v_w"][l].astype(np.float32)
            gw = ln2[l][:, None] * inp["gate_w"][l].astype(np.float32)
            uw = ln2[l][:, None] * inp["up_w"][l].astype(np.float32)
            dw = inp["down_w"][l].astype(np.float32)
            gws = np.zeros((H, IP), np.float32)
            uws = np.zeros((H, IP), np.float32)
            dws = np.zeros((IP, H), np.float32)
            gws[:, :IPC] = gw[:, c * IPC:(c + 1) * IPC]
            uws[:, :IPC] = uw[:, c * IPC:(c + 1) * IPC]
            dws[:IPC] = dw[c * IPC:(c + 1) * IPC]
            mcore[f"qw{l}"] = np.ascontiguousarray(qw[:, c * 512:(c + 1) * 512]).astype(bf16)
            mcore[f"kvw{l}"] = np.concatenate(
                [kw[:, c * 128:(c + 1) * 128], vw[:, c * 128:(c + 1) * 128]],
                1).astype(bf16)
            mcore[f"ow{l}"] = np.ascontiguousarray(
                inp["o_w"][l][c * 512:(c + 1) * 512].astype(np.float32)).astype(bf16)
            mcore[f"gw{l}"] = gws.astype(bf16)
            mcore[f"uw{l}"] = uws.astype(bf16)
            mcore[f"dw{l}"] = dws.astype(bf16)
        lmc = lm_folded[:, c * VS:(c + 1) * VS]
        mcore["lmw"] = np.ascontiguousarray(
            lmc.reshape(H, 8, VS // 8).transpose(1, 0, 2)).astype(bf16)
        in_maps.append(mcore)

    return in_maps, valid_len


def kernel(**inputs) -> np.ndarray:
    in_maps, valid_len = host_prep(inputs)
    if "nc" not in _cache:
        _cache["nc"] = build_nc()
    nc = _cache["nc"]
    res = run_bass_kernel_spmd(nc, in_maps, list(range(NC_)),
                               **last_run_info.get("run_kwargs", {}))
    last_run_info["res"] = res
    out = res.results[0]
    gmax = out["gmax_o"].transpose(1, 0).reshape(S).astype(np.float64)
    gsum = out["gsum_o"].transpose(1, 0).reshape(S).astype(np.float64)
    tlog = out["tlog_o"].reshape(S).astype(np.float64)
    ce = gmax + np.log(gsum) - tlog
    w = (np.arange(S - 1) < valid_len - 1).astype(np.float64)
    loss = (ce[:S - 1] * w).sum() / w.sum()
    return np.float32(loss)


# revision 13
# speedup vs baseline: 1.0431x; 1.0008x over previous
"""Trainium2 Bass kernel for nn_Decoder_20486994002617.

8-core tensor-parallel 2-layer llama-style decoder with ragged token-merge
(handled on host), returning the masked-mean cross-entropy loss.

Device layout choices:
  - h (residual) lives in SBUF as [128 part, 8 seq-tiles, 4096] bf16.
  - RMSNorm weights are folded into the consumer weight matrices on host,
    so the device norm is x * rsqrt(mean(x^2)+eps) only; the multiply by
    the per-row factor is fused into the seq->feature transpose as a
    matmul against diag(factor).
  - Attention: heads sharded 4 q-heads + 1 kv-head per core (GQA groups
    align), scores/softmax per (head, 128-row tile), causal mask added via
    an extra accumulating matmul (I.T @ cmask), attn probs transposed back
    through the PE with diag(1/sumexp) fused.
  - MLP: intermediate dim sharded 1376/core, padded to 1408 = 11*128.
  - lm_head: vocab sharded 4000/core; softmax stats (row max, sum-exp) are
    AllReduce'd; the target logit is computed via a host-gathered column
    matrix (wsel) so no device gather is needed.
Outputs per core: gmax [128,8] f32, gsum [128,8] f32, tlog [1,1024] f32.
Host finishes: ce = gmax + log(gsum) - tlog; loss = masked mean.
"""
import numpy as np
import ml_dtypes

from contextlib import ExitStack

import concourse.bass as bass
import concourse.bacc as bacc
import concourse.mybir as mybir
import concourse.tile as tile
from concourse.bass_utils import run_bass_kernel_spmd

F32 = mybir.dt.float32
BF16 = mybir.dt.bfloat16
AF = mybir.ActivationFunctionType
ALU = mybir.AluOpType
AX = mybir.AxisListType

H, HD, NH, NKV = 4096, 128, 32, 8
L, V, S, I = 2, 32000, 1024, 11008
EPS, THETA = 1e-6, 10000.0
NC_ = 8          # cores
IPC = I // NC_   # 1376
IP = 1408        # padded intermediate per core = 11 * 128
VS = V // NC_    # 4000 vocab per core
NEG = -1e9

bf16 = ml_dtypes.bfloat16

last_run_info = {}
_cache = {}


# ----------------------------------------------------------------- device --

def _norm_transpose(nc, pools, h_ap, dst, ident_sb, uid):
    """dst[:, k, :] (32 chunks of [128,128]) = normalized transpose of
    h_ap ([128 seq rows, 4096]). dst free dims must be (32, 128)."""
    small, ntmp, psum = pools
    sq = ntmp.tile([128, 4096], BF16, tag="nt_sq", bufs=1, name=f"sq_{uid}")
    ssq = small.tile([128, 1], F32, tag="nt_ssq", bufs=2, name=f"ssq_{uid}")
    nc.scalar.activation(sq[:], h_ap, AF.Square, accum_out=ssq[:])
    var = small.tile([128, 1], F32, tag="nt_var", bufs=2, name=f"var_{uid}")
    nc.vector.tensor_scalar(var[:], ssq[:], 1.0 / H, EPS, op0=ALU.mult, op1=ALU.add)
    std = small.tile([128, 1], F32, tag="nt_std", bufs=2, name=f"std_{uid}")
    nc.scalar.sqrt(std[:], var[:])
    fac = small.tile([128, 1], F32, tag="nt_fac", bufs=2, name=f"fac_{uid}")
    nc.vector.reciprocal(fac[:], std[:])
    diag = ntmp.tile([128, 128], BF16, tag="nt_diag", bufs=2, name=f"diag_{uid}")
    nc.vector.tensor_scalar_mul(diag[:], ident_sb[:], fac[:])
    for kk in range(8):
        pnt = psum.tile([128, 512], F32, tag="nt_ps", bufs=2, name=f"pnt_{uid}_{kk}")
        for j in range(4):
            k = kk * 4 + j
            nc.tensor.matmul(pnt[:, j * 128:(j + 1) * 128],
                             h_ap[:, k * 128:(k + 1) * 128], diag[:],
                             start=True, stop=True)
        nc.any.tensor_copy(dst[:, kk * 4:(kk + 1) * 4, :],
                           pnt[:].rearrange("p (j m) -> p j m", j=4))


def _rope(nc, pools, ps, cos_ap, sf_ap, out, nheads, i):
    """out (bf16 [128, nheads*128]) = rope(ps) with ps a psum slice."""
    small, ntmp, psum = pools
    n = nheads * 128
    t1 = ntmp.tile([128, 512], F32, tag="rope_t1", bufs=2, name=f"t1_{i}_{nheads}")
    t2 = ntmp.tile([128, 512], F32, tag="rope_t2", bufs=2, name=f"t2_{i}_{nheads}")
    nc.vector.tensor_mul(t1[:, :n], ps, cos_ap)
    for hh in range(nheads):
        b = hh * 128
        nc.vector.tensor_mul(t2[:, b:b + 64], ps[:, b + 64:b + 128],
                             sf_ap[:, b:b + 64])
        nc.vector.tensor_mul(t2[:, b + 64:b + 128], ps[:, b:b + 64],
                             sf_ap[:, b + 64:b + 128])
    nc.vector.tensor_add(out[:], t1[:, :n], t2[:, :n])


def build_nc():
    nc = bacc.Bacc("TRN2", target_bir_lowering=False, debug=False,
                   num_devices=NC_)

    din = {}
    def dram_in(name, shape):
        din[name] = nc.dram_tensor(name, shape, BF16, kind="ExternalInput")
        return din[name]

    h0_d = dram_in("h0", [S, H])
    cos4_d = dram_in("cos4", [S, 512])
    sf4_d = dram_in("sf4", [S, 512])
    ident_d = dram_in("ident", [128, 128])
    cmask_d = dram_in("cmask", [128, 128])
    ones_d = dram_in("ones", [128, 1])
    for l in range(L):
        dram_in(f"qw{l}", [H, 512])
        dram_in(f"kvw{l}", [H, 256])
        dram_in(f"ow{l}", [512, H])
        dram_in(f"gw{l}", [H, IP])
        dram_in(f"uw{l}", [H, IP])
        dram_in(f"dw{l}", [IP, H])
    lmw_d = dram_in("lmw", [8, H, VS // 8])
    wsel_d = dram_in("wsel", [H, S])

    gmax_o = nc.dram_tensor("gmax_o", [128, 8], F32, kind="ExternalOutput")
    gsum_o = nc.dram_tensor("gsum_o", [128, 8], F32, kind="ExternalOutput")
    tlog_o = nc.dram_tensor("tlog_o", [1, S], F32, kind="ExternalOutput")

    rg = [list(range(NC_))]

    with tile.TileContext(nc) as tc:
        with (
            tc.tile_pool(name="pconst", bufs=1) as pconst,
            tc.tile_pool(name="psmall", bufs=1) as psmall,
            tc.tile_pool(name="pdram", bufs=1, space="DRAM") as pdram,
        ):
            ident_sb = pconst.tile([128, 128], BF16)
            cmask_sb = pconst.tile([128, 128], BF16)
            ones_sb = pconst.tile([128, 1], BF16)
            cos4_sb = pconst.tile([128, 8, 512], BF16)
            sf4_sb = pconst.tile([128, 8, 512], BF16)
            nc.sync.dma_start(ident_sb[:], ident_d.ap())
            nc.sync.dma_start(cmask_sb[:], cmask_d.ap())
            nc.sync.dma_start(ones_sb[:], ones_d.ap())
            for i in range(8):
                nc.sync.dma_start(cos4_sb[:, i, :], cos4_d.ap()[i * 128:(i + 1) * 128, :])
                nc.sync.dma_start(sf4_sb[:, i, :], sf4_d.ap()[i * 128:(i + 1) * 128, :])

            hstack = ExitStack()
            phh = hstack.enter_context(tc.tile_pool(name="phh", bufs=1))
            if True:
                h_sb = phh.tile([128, 8, H], BF16)
                for i in range(8):
                    nc.sync.dma_start(h_sb[:, i, :], h0_d.ap()[i * 128:(i + 1) * 128, :])

                for l in range(L):
                    # ============================== attention ==============
                    with (
                        tc.tile_pool(name="pal", bufs=1) as pal,      # attn-long
                        tc.tile_pool(name="pdr", bufs=1, space="DRAM") as pdr,
                    ):
                        qT_sb = pal.tile([128, 4, S], BF16)
                        kT_sb = pal.tile([128, S], BF16)
                        v_sb = pal.tile([128, 8, 128], BF16)
                        oT_sb = pal.tile([128, 4, S], BF16)
                        ar_in = pdr.tile([S, H], BF16)
                        ar_outs = [pdr.tile([512, H], BF16, addr_space="Shared",
                                            name=f"ar_out_{l}_{c}") for c in range(2)]

                        with (
                            tc.tile_pool(name="pqkv", bufs=1) as pqkv,
                            tc.tile_pool(name="pqps", bufs=1, space="PSUM") as pqps,
                        ):
                            pools = (psmall, pqkv, pqps)
                            wq_sb = pqkv.tile([128, 32, 512], BF16)
                            wkv_sb = pqkv.tile([128, 32, 256], BF16)
                            nc.sync.dma_start(
                                wq_sb[:], din[f"qw{l}"].ap().rearrange("(k p) n -> p k n", p=128))
                            nc.sync.dma_start(
                                wkv_sb[:], din[f"kvw{l}"].ap().rearrange("(k p) n -> p k n", p=128))
                            for i in range(8):
                                xnt = pqkv.tile([128, 32, 128], BF16, tag="xnt",
                                                bufs=2, name=f"xnt_{l}_{i}")
                                _norm_transpose(nc, pools, h_sb[:, i, :], xnt, ident_sb, f'a{l}_{i}')
                                psq = pqps.tile([128, 512], F32, tag="psq", bufs=2,
                                                name=f"psq_{l}_{i}")
                                pskv = pqps.tile([128, 256], F32, tag="pskv", bufs=2,
                                                 name=f"pskv_{l}_{i}")
                                for k in range(32):
                                    nc.tensor.matmul(psq[:], xnt[:, k, :], wq_sb[:, k, :],
                                                     start=(k == 0), stop=(k == 31))
                                    nc.tensor.matmul(pskv[:], xnt[:, k, :], wkv_sb[:, k, :],
                                                     start=(k == 0), stop=(k == 31))
                                q_rot = pqkv.tile([128, 512], BF16, tag="q_rot", bufs=2,
                                                  name=f"qr_{l}_{i}")
                                k_rot = pqkv.tile([128, 128], BF16, tag="k_rot", bufs=2,
                                                  name=f"kr_{l}_{i}")
                                _rope(nc, pools, psq[:], cos4_sb[:, i, :], sf4_sb[:, i, :],
                                      q_rot, 4, i)
                                _rope(nc, pools, pskv[:, 0:128], cos4_sb[:, i, 0:128],
                                      sf4_sb[:, i, 0:128], k_rot, 1, i)
                                nc.any.tensor_copy(v_sb[:, i, :], pskv[:, 128:256])
                                for hh in range(4):
                                    ptr = pqps.tile([128, 128], F32, tag="ptr", bufs=2,
                                                    name=f"ptrq_{l}_{i}_{hh}")
                                    nc.tensor.matmul(ptr[:], q_rot[:, hh * 128:(hh + 1) * 128],
                                                     ident_sb[:], start=True, stop=True)
                                    nc.any.tensor_copy(qT_sb[:, hh, i * 128:(i + 1) * 128], ptr[:])
                                ptrk = pqps.tile([128, 128], F32, tag="ptr", bufs=2,
                                                 name=f"ptrk_{l}_{i}")
                                nc.tensor.matmul(ptrk[:], k_rot[:], ident_sb[:],
                                                 start=True, stop=True)
                                nc.any.tensor_copy(kT_sb[:, i * 128:(i + 1) * 128], ptrk[:])

                        with (
                            tc.tile_pool(name="phd", bufs=1) as phd,
                            tc.tile_pool(name="phps", bufs=1, space="PSUM") as phps,
                        ):
                            for hh in range(4):
                                for i in range(8):
                                    n2 = 128 * (i + 1)
                                    pss = phps.tile([128, 1024], F32, tag="pss", bufs=2,
                                                    name=f"pss_{l}_{hh}_{i}")
                                    lhs_q = qT_sb[:, hh, i * 128:(i + 1) * 128]
                                    c0 = 0
                                    while c0 < n2 - 128:
                                        N = min(512, n2 - 128 - c0)
                                        nc.tensor.matmul(pss[:, c0:c0 + N], lhs_q,
                                                         kT_sb[:, c0:c0 + N],
                                                         start=True, stop=True)
                                        c0 += N
                                    nc.tensor.matmul(pss[:, n2 - 128:n2], lhs_q,
                                                     kT_sb[:, n2 - 128:n2],
                                                     start=True, stop=False)
                                    nc.tensor.matmul(pss[:, n2 - 128:n2], ident_sb[:],
                                                     cmask_sb[:], start=False, stop=True)
                                    mx = psmall.tile([128, 1], F32, tag="mx", bufs=2,
                                                     name=f"mx_{l}_{hh}_{i}")
                                    nc.vector.tensor_reduce(mx[:], pss[:, :n2], axis=AX.X,
                                                            op=ALU.max)
                                    negm = psmall.tile([128, 1], F32, tag="negm", bufs=2,
                                                       name=f"negm_{l}_{hh}_{i}")
                                    nc.vector.tensor_scalar_mul(negm[:], mx[:], -1.0)
                                    sume = psmall.tile([128, 1], F32, tag="sume", bufs=2,
                                                       name=f"sume_{l}_{hh}_{i}")
                                    exp_sb = phd.tile([128, 1024], BF16, tag="exp", bufs=2,
                                                      name=f"exp_{l}_{hh}_{i}")
                                    nc.scalar.activation(exp_sb[:, :n2], pss[:, :n2], AF.Exp,
                                                         bias=negm[:], accum_out=sume[:])
                                    rec = psmall.tile([128, 1], F32, tag="rec", bufs=2,
                                                      name=f"rec_{l}_{hh}_{i}")
                                    nc.vector.reciprocal(rec[:], sume[:])
                                    diag_r = phd.tile([128, 128], BF16, tag="diag_r", bufs=2,
                                                      name=f"diagr_{l}_{hh}_{i}")
                                    nc.vector.tensor_scalar_mul(diag_r[:], ident_sb[:], rec[:])
                                    atcol = phd.tile([128, 8, 128], BF16, tag="atcol", bufs=2,
                                                     name=f"atcol_{l}_{hh}_{i}")
                                    for j in range(i + 1):
                                        pat = phps.tile([128, 128], F32, tag="pat", bufs=2,
                                                        name=f"pat_{l}_{hh}_{i}_{j}")
                                        nc.tensor.matmul(pat[:], exp_sb[:, j * 128:(j + 1) * 128],
                                                         diag_r[:], start=True, stop=True)
                                        nc.any.tensor_copy(atcol[:, j, :], pat[:])
                                    pso = phps.tile([128, 128], F32, tag="pso", bufs=2,
                                                    name=f"pso_{l}_{hh}_{i}")
                                    for j in range(i + 1):
                                        nc.tensor.matmul(pso[:], v_sb[:, j, :], atcol[:, j, :],
                                                         start=(j == 0), stop=(j == i))
                                    nc.any.tensor_copy(oT_sb[:, hh, i * 128:(i + 1) * 128], pso[:])

                        with (
                            tc.tile_pool(name="pop", bufs=1) as pop,
                            tc.tile_pool(name="pops", bufs=1, space="PSUM") as pops,
                        ):
                            ow_sb = pop.tile([128, 4, H], BF16)
                            nc.sync.dma_start(
                                ow_sb[:], din[f"ow{l}"].ap().rearrange("(t p) n -> p t n", p=128))
                            for i in range(8):
                                ob = pop.tile([128, H], BF16, tag="ob", bufs=3,
                                              name=f"ob_{l}_{i}")
                                for n in range(8):
                                    pps = pops.tile([128, 512], F32, tag="pop", bufs=4,
                                                    name=f"pop_{l}_{i}_{n}")
                                    for t in range(4):
                                        nc.tensor.matmul(pps[:], oT_sb[:, t, i * 128:(i + 1) * 128],
                                                         ow_sb[:, t, n * 512:(n + 1) * 512],
                                                         start=(t == 0), stop=(t == 3))
                                    nc.any.tensor_copy(ob[:, n * 512:(n + 1) * 512], pps[:])
                                nc.sync.dma_start(
                                    ar_in[i * 128:(i + 1) * 128, :], ob[:])
                                if i == 3:
                                    nc.gpsimd.collective_compute(
                                        "AllReduce", ALU.add, replica_groups=rg,
                                        ins=[ar_in[0:512, :].opt()], outs=[ar_outs[0].opt()])
                            nc.gpsimd.collective_compute(
                                "AllReduce", ALU.add, replica_groups=rg,
                                ins=[ar_in[512:1024, :].opt()], outs=[ar_outs[1].opt()])

                        with tc.tile_pool(name="pres", bufs=1) as pres:
                            for half in range(2):
                                for ii in range(4):
                                    i = half * 4 + ii
                                    rt = pres.tile([128, H], BF16, tag="res", bufs=3,
                                                   name=f"res_{l}_{i}")
                                    nc.gpsimd.dma_start(
                                        rt[:], ar_outs[half][ii * 128:(ii + 1) * 128, :])
                                    nc.vector.tensor_add(h_sb[:, i, :], h_sb[:, i, :],
                                                         rt[:])

                    # ============================== MLP ====================
                    with (
                        tc.tile_pool(name="pml", bufs=1) as pml,
                        tc.tile_pool(name="pdr2", bufs=1, space="DRAM") as pdr2,
                    ):
                        yt_sb = pml.tile([128, 11, S], BF16)
                        ar2_in = pdr2.tile([S, H], BF16)
                        ar2_outs = [pdr2.tile([512, H], BF16, addr_space="Shared",
                                              name=f"ar2_out_{l}_{c}") for c in range(2)]

                        for ig in range(2):
                            with (
                                tc.tile_pool(name="pgu", bufs=1) as pgu,
                                tc.tile_pool(name="pgps", bufs=1, space="PSUM") as pgps,
                            ):
                                pools = (psmall, pgu, pgps)
                                xnts = []
                                for ii in range(4):
                                    i = ig * 4 + ii
                                    xnt = pgu.tile([128, 32, 128], BF16, tag="xnt2",
                                                   bufs=4, name=f"xnt2_{l}_{i}")
                                    _norm_transpose(nc, pools, h_sb[:, i, :], xnt, ident_sb, f'a{l}_{i}')
                                    xnts.append(xnt)
                                gu = {}
                                for wname, tag in ((f"gw{l}", "g"), (f"uw{l}", "u")):
                                    outs = [pgu.tile([128, IP], BF16, tag=tag, bufs=4,
                                                     name=f"{tag}_{l}_{ig}_{ii}")
                                            for ii in range(4)]
                                    gu[tag] = outs
                                    for nb in range(3):
                                        NB = 512 if nb < 2 else IP - 1024
                                        pg = [pgps.tile([128, 512], F32, tag="pg", bufs=4,
                                                        name=f"pg_{l}_{ig}_{tag}_{nb}_{ii}")
                                              for ii in range(4)]
                                        for kp in range(8):
                                            wt = pgu.tile([128, 4, 512], BF16, tag="wstream",
                                                          bufs=4,
                                                          name=f"wt_{l}_{ig}_{tag}_{nb}_{kp}")
                                            nc.sync.dma_start(
                                                wt[:, :, :NB],
                                                din[wname].ap()[kp * 512:(kp + 1) * 512,
                                                                nb * 512:nb * 512 + NB]
                                                .rearrange("(j p) n -> p j n", p=128))
                                            for jk in range(4):
                                                k = kp * 4 + jk
                                                for ii in range(4):
                                                    nc.tensor.matmul(pg[ii][:, :NB],
                                                                     xnts[ii][:, k, :],
                                                                     wt[:, jk, :NB],
                                                                     start=(k == 0), stop=(k == 31))
                                        for ii in range(4):
                                            nc.any.tensor_copy(
                                                outs[ii][:, nb * 512:nb * 512 + NB],
                                                pg[ii][:, :NB])
                                for ii in range(4):
                                    i = ig * 4 + ii
                                    ysil = pgu.tile([128, IP], BF16, tag="ysil", bufs=2,
                                                    name=f"ysil_{l}_{i}")
                                    nc.scalar.activation(ysil[:], gu["g"][ii][:], AF.Silu)
                                    y = pgu.tile([128, IP], BF16, tag="y", bufs=2,
                                                 name=f"y_{l}_{i}")
                                    nc.vector.tensor_mul(y[:], ysil[:], gu["u"][ii][:])
                                    for tq in range(3):
                                        ts = [tq * 4 + j for j in range(4) if tq * 4 + j < 11]
                                        ptr = pgps.tile([128, 512], F32, tag="ytr", bufs=2,
                                                        name=f"ytr_{l}_{i}_{tq}")
                                        for jj, t in enumerate(ts):
                                            nc.tensor.matmul(ptr[:, jj * 128:(jj + 1) * 128],
                                                             y[:, t * 128:(t + 1) * 128],
                                                             ident_sb[:], start=True, stop=True)
                                        nc.any.tensor_copy(
                                            yt_sb[:, ts[0]:ts[0] + len(ts),
                                                  i * 128:(i + 1) * 128],
                                            ptr[:, :len(ts) * 128].rearrange(
                                                "p (j m) -> p j m", j=len(ts)))

                        with (
                            tc.tile_pool(name="pdn", bufs=1) as pdn,
                            tc.tile_pool(name="pdps", bufs=1, space="PSUM") as pdps,
                        ):
                            for half in range(2):
                                dbs = [pdn.tile([128, H], BF16, tag=f"db{ii}", bufs=2,
                                                name=f"db_{l}_{half}_{ii}")
                                       for ii in range(4)]
                                for n in range(8):
                                    pd = [pdps.tile([128, 512], F32, tag=f"pd{ii}", bufs=2,
                                                    name=f"pd_{l}_{half}_{n}_{ii}")
                                          for ii in range(4)]
                                    for tp in range(3):
                                        nt = 4 if tp < 2 else 3
                                        dwt = pdn.tile([128, 4, 512], BF16, tag="dwstream",
                                                       bufs=4, name=f"dwt_{l}_{half}_{n}_{tp}")
                                        nc.sync.dma_start(
                                            dwt[:, :nt, :],
                                            din[f"dw{l}"].ap()[tp * 512:tp * 512 + nt * 128,
                                                               n * 512:(n + 1) * 512]
                                            .rearrange("(j p) n -> p j n", p=128))
                                        for jt in range(nt):
                                            t = tp * 4 + jt
                                            for ii in range(4):
                                                i = half * 4 + ii
                                                nc.tensor.matmul(
                                                    pd[ii][:], yt_sb[:, t, i * 128:(i + 1) * 128],
                                                    dwt[:, jt, :], start=(t == 0), stop=(t == 10))
                                    for ii in range(4):
                                        nc.any.tensor_copy(dbs[ii][:, n * 512:(n + 1) * 512],
                                                           pd[ii][:])
                                for ii in range(4):
                                    i = half * 4 + ii
                                    nc.sync.dma_start(ar2_in[i * 128:(i + 1) * 128, :],
                                                      dbs[ii][:])
                                nc.gpsimd.collective_compute(
                                    "AllReduce", ALU.add, replica_groups=rg,
                                    ins=[ar2_in[half * 512:(half + 1) * 512, :].opt()],
                                    outs=[ar2_outs[half].opt()])

                        with tc.tile_pool(name="pres2", bufs=1) as pres2:
                            for half in range(2):
                                for ii in range(4):
                                    i = half * 4 + ii
                                    rt = pres2.tile([128, H], BF16, tag="res2", bufs=3,
                                                    name=f"res2_{l}_{i}")
                                    nc.gpsimd.dma_start(
                                        rt[:], ar2_outs[half][ii * 128:(ii + 1) * 128, :])
                                    nc.vector.tensor_add(h_sb[:, i, :], h_sb[:, i, :],
                                                         rt[:])

                # spill h to DRAM so the h pool can close before lm phase
                hdram = pdram.tile([S, H], BF16)
                for i in range(8):
                    nc.gpsimd.dma_start(hdram[i * 128:(i + 1) * 128, :], h_sb[:, i, :])
            hstack.close()  # release h pool

            # ======================= final norm -> xf ======================
            with tc.tile_pool(name="pxf", bufs=1) as pxf:
                xf_sb = pxf.tile([128, 32, S], BF16)
                with (
                    tc.tile_pool(name="pfn", bufs=1) as pfn,
                    tc.tile_pool(name="pfps", bufs=1, space="PSUM") as pfps,
                ):
                    pools = (psmall, pfn, pfps)
                    for i in range(8):
                        ht = pfn.tile([128, H], BF16, tag="hfin", bufs=2,
                                      name=f"hfin_{i}")
                        nc.gpsimd.dma_start(ht[:], hdram[i * 128:(i + 1) * 128, :])
                        dst = xf_sb[:, :, i * 128:(i + 1) * 128]
                        _norm_transpose(nc, pools, ht[:], dst, ident_sb, f"f{i}")
                self_lm_phases(nc, tc, psmall, xf_sb, ident_sb, ones_sb,
                               wsel_d, lmw_d, tlog_o, gmax_o, gsum_o, rg)

    nc.compile()
    return nc


def self_lm_phases(nc, tc, psmall, xf_sb, ident_sb, ones_sb, wsel_d, lmw_d,
                   tlog_o, gmax_o, gsum_o, rg):
            if True:
                pass
            with (
                tc.tile_pool(name="ptl", bufs=1) as ptl,
                tc.tile_pool(name="ptps", bufs=1, space="PSUM") as ptps,
            ):
                pt0 = ptps.tile([1, 512], F32)
                pt1 = ptps.tile([1, 512], F32)
                for kp in range(8):
                    ws = ptl.tile([128, 4, S], BF16, tag="wsel", bufs=2, name=f"ws_{kp}")
                    nc.sync.dma_start(
                        ws[:], wsel_d.ap()[kp * 512:(kp + 1) * 512, :]
                        .rearrange("(j p) n -> p j n", p=128))
                    for jk in range(4):
                        k = kp * 4 + jk
                        tm = ptl.tile([128, S], BF16, tag="tm", bufs=2, name=f"tm_{k}")
                        nc.vector.tensor_mul(tm[:], xf_sb[:, k, :], ws[:, jk, :])
                        nc.tensor.matmul(pt0[:], ones_sb[:], tm[:, :512],
                                         start=(k == 0), stop=(k == 31))
                        nc.tensor.matmul(pt1[:], ones_sb[:], tm[:, 512:],
                                         start=(k == 0), stop=(k == 31))
                tl_sb = ptl.tile([1, S], F32)
                nc.any.tensor_copy(tl_sb[:, :512], pt0[:])
                nc.any.tensor_copy(tl_sb[:, 512:], pt1[:])
                nc.sync.dma_start(tlog_o.ap(), tl_sb[:])

            with (
                tc.tile_pool(name="plm", bufs=1) as plm,
                tc.tile_pool(name="plps", bufs=1, space="PSUM") as plps,
                tc.tile_pool(name="pld", bufs=1, space="DRAM") as pld,
            ):
                logits = [plm.tile([128, VS], BF16, tag=f"lg{i}", bufs=1,
                                   name=f"logits_{i}") for i in range(8)]
                for vb in range(8):
                    pl = [plps.tile([128, 500], F32, tag=f"pl{i}", bufs=1,
                                    name=f"pl_{vb}_{i}") for i in range(8)]
                    for kp in range(8):
                        lt = plm.tile([128, 4, 500], BF16, tag="lmw", bufs=4,
                                      name=f"lt_{vb}_{kp}")
                        nc.sync.dma_start(
                            lt[:], lmw_d.ap()[vb, kp * 512:(kp + 1) * 512, :]
                            .rearrange("(j p) n -> p j n", p=128))
                        for jk in range(4):
                            k = kp * 4 + jk
                            for i in range(8):
                                nc.tensor.matmul(pl[i][:], xf_sb[:, k, i * 128:(i + 1) * 128],
                                                 lt[:, jk, :], start=(k == 0), stop=(k == 31))
                    for i in range(8):
                        nc.any.tensor_copy(logits[i][:, vb * 500:(vb + 1) * 500], pl[i][:])

                gmax_sb = plm.tile([128, 8], F32)
                for i in range(8):
                    nc.vector.tensor_reduce(gmax_sb[:, i:i + 1], logits[i][:],
                                            axis=AX.X, op=ALU.max)
                gm_in = pld.tile([128, 8], F32)
                gm_out = pld.tile([128, 8], F32, addr_space="Shared")
                nc.sync.dma_start(gm_in[:], gmax_sb[:])
                nc.gpsimd.collective_compute("AllReduce", ALU.max, replica_groups=rg,
                                             ins=[gm_in.opt()], outs=[gm_out.opt()])
                gm_sb = plm.tile([128, 8], F32)
                nc.sync.dma_start(gm_sb[:], gm_out[:])
                nc.sync.dma_start(gmax_o.ap(), gm_sb[:])
                negg = plm.tile([128, 8], F32)
                nc.vector.tensor_scalar_mul(negg[:], gm_sb[:], -1.0)
                gs_sb = plm.tile([128, 8], F32)
                for i in range(8):
                    scr = plm.tile([128, VS], BF16, tag="scr", bufs=2, name=f"scr_{i}")
                    nc.scalar.activation(scr[:], logits[i][:], AF.Exp,
                                         bias=negg[:, i:i + 1],
                                         accum_out=gs_sb[:, i:i + 1])
                gs_in = pld.tile([128, 8], F32)
                gs_out = pld.tile([128, 8], F32, addr_space="Shared")
                nc.sync.dma_start(gs_in[:], gs_sb[:])
                nc.gpsimd.collective_compute("AllReduce", ALU.add, replica_groups=rg,
                                             ins=[gs_in.opt()], outs=[gs_out.opt()])
                gsf_sb = plm.tile([128, 8], F32)
                nc.sync.dma_start(gsf_sb[:], gs_out[:])
                nc.sync.dma_start(gsum_o.ap(), gsf_sb[:])


# ------------------------------------------------------------------- host --

def host_prep(inputs):
    inp = {k: np.asarray(v) for k, v in inputs.items()}
    embed = inp["embed"].astype(np.float32)
    ids = inp["input_ids"].reshape(-1).astype(np.int64)
    labels = inp["labels"].reshape(-1).astype(np.int64)

    h = embed[ids]
    cw = inp["conv_w"].astype(np.float32)
    logit = h[:-1] @ cw[0, :H] + h[1:] @ cw[0, H:] + np.float32(inp["conv_b"][0])
    mask = logit > 0
    m = np.concatenate([mask, [False]])
    hn = np.where(m[:, None], 0.5 * (h + np.roll(h, -1, axis=0)), h)
    keep = np.concatenate([[True], ~mask])
    order = np.argsort(~keep, kind="stable")
    h0 = hn[order]
    lab = labels[order]
    valid_len = int(keep.sum())

    inv = 1.0 / (THETA ** (np.arange(0, HD, 2, dtype=np.float32) / HD))
    t = np.arange(S, dtype=np.float32)
    freqs = np.outer(t, inv)
    emb = np.concatenate([freqs, freqs], -1)
    cos, sin = np.cos(emb), np.sin(emb)
    sinflip = np.concatenate([-sin[:, :HD // 2], sin[:, HD // 2:]], -1)
    cos4 = np.tile(cos, (1, 4)).astype(bf16)
    sinflip4 = np.tile(sinflip, (1, 4)).astype(bf16)

    ident = np.eye(128, dtype=bf16)
    cmask = np.where(np.arange(128)[None, :] > np.arange(128)[:, None],
                     np.float32(NEG), np.float32(0)).astype(bf16)
    ones = np.ones((128, 1), dtype=bf16)

    ln1 = inp["ln1_w"].astype(np.float32)
    ln2 = inp["ln2_w"].astype(np.float32)
    normw = inp["norm_w"].astype(np.float32)
    qsc = np.float32(1.0 / np.sqrt(HD))
    lm_folded = normw[:, None] * inp["lm_head_w"].astype(np.float32)
    tgt = np.concatenate([lab[1:], [0]]).astype(np.int64)
    wsel = np.ascontiguousarray(lm_folded[:, tgt]).astype(bf16)

    common = dict(h0=h0.astype(bf16), cos4=cos4, sf4=sinflip4, ident=ident,
                  cmask=cmask, ones=ones, wsel=wsel)
    in_maps = []
    for c in range(NC_):
        mcore = dict(common)
        for l in range(L):
            qw = ln1[l][:, None] * inp["q_w"][l].astype(np.float32) * qsc
            kw = ln1[l][:, None] * inp["k_w"][l].astype(np.float32)
            vw = ln1[l][:, None] * inp["v_w"][l].astype(np.float32)
            gw = ln2[l][:, None] * inp["gate_w"][l].astype(np.float32)
            uw = ln2[l][:, None] * inp["up_w"][l].astype(np.float32)
            dw = inp["down_w"][l].astype(np.float32)
            gws = np.zeros((H, IP), np.float32)
            uws = np.zeros((H, IP), np.float32)
            dws = np.zeros((IP, H), np.float32)
            gws[:, :IPC] = gw[:, c * IPC:(c + 1) * IPC]
            uws[:, :IPC] = uw[:, c * IPC:(c + 1) * IPC]
            dws[:IPC] = dw[c * IPC:(c + 1) * IPC]
            mcore[f"qw{l}"] = np.ascontiguousarray(qw[:, c * 512:(c + 1) * 512]).astype(bf16)
            mcore[f"kvw{l}"] = np.concatenate(
                [kw[:, c * 128:(c + 1) * 128], vw[:, c * 128:(c + 1) * 128]],
                1).astype(bf16)
            mcore[f"ow{l}"] = np.ascontiguousarray(
                inp["o_w"][l][c * 512:(c + 1) * 512].astype(np.float32)).astype(bf16)
            mcore[f"gw{l}"] = gws.astype(bf16)
            mcore[f"uw{l}"] = uws.astype(bf16)
            mcore[f"dw{l}"] = dws.astype(bf16)
        lmc = lm_folded[:, c * VS:(c + 1) * VS]
        mcore["lmw"] = np.ascontiguousarray(
            lmc.reshape(H, 8, VS // 8).transpose(1, 0, 2)).astype(bf16)
        in_maps.append(mcore)

    return in_maps, valid_len


def kernel(**inputs) -> np.ndarray:
    in_maps, valid_len = host_prep(inputs)
    if "nc" not in _cache:
        _cache["nc"] = build_nc()
    nc = _cache["nc"]
    res = run_bass_kernel_spmd(nc, in_maps, list(range(NC_)),
                               **last_run_info.get("run_kwargs", {}))
    last_run_info["res"] = res
    out = res.results[0]
    gmax = out["gmax_o"].transpose(1, 0).reshape(S).astype(np.float64)
    gsum = out["gsum_o"].transpose(1, 0).reshape(S).astype(np.float64)
    tlog = out["tlog_o"].reshape(S).astype(np.float64)
    ce = gmax + np.log(gsum) - tlog
    w = (np.arange(S - 1) < valid_len - 1).astype(np.float64)
    loss = (ce[:S - 1] * w).sum() / w.sum()
    return np.float32(loss)
